# revision 17
# baseline (speedup 1.0000x reference)
"""Trainium2 Bass kernel for local (windowed causal) self-attention.

Problem: B=2, T=2048, C=1024, 16 heads x 64 dim, local window 256.
Sharding: T-sharding. 8 cores = 2 batches x 4 chunks of 512 tokens.
Each core receives its 512-token chunk plus a 256-token left halo of x,
pre-transposed on the host to x^T (zero-padded for chunk 0), computes
QKV / banded attention / output projection for its own rows, and writes
a disjoint [512, 1024] slice of the output. No collectives; the host
concatenates the 8 slices.

Self-contained: hardcodes all shapes; no reads of /root/problem/*.
"""

import os

os.environ.setdefault("MYCRO_LOCAL_CACHE", "1")

import numpy as np

# ---------------------------------------------------------------- constants
B, T, C = 2, 2048, 1024
H, D = 16, 64
WIN = 256                      # local attention context
NCORES = 8
CHUNK = 512                    # queries per core
HALO = 256                     # left halo (== WIN)
TQ = CHUNK + HALO              # 768 x rows per core
P = 128

NQT = CHUNK // P               # 4 query tiles per core
NKT = TQ // P                  # 6 key tiles per core

# (kt, qt) pairs whose exp'd slab block needs a multiplicative 0/1 mask.
# kt-qt==2 -> window edge; kt-qt==0 -> causal edge; (1,0) is all-valid
# generically but fully invalid on the boundary chunk (keys < 0), included
# so every core runs an identical instruction stream.
MASK_PAIRS = [(0, 0), (1, 1), (2, 2), (3, 3),
              (1, 0),
              (2, 0), (3, 1), (4, 2), (5, 3)]
NMASK = len(MASK_PAIRS)

_MODS = {}                     # cached compiled Bass modules


def _np_bf16():
    import ml_dtypes
    return np.dtype(ml_dtypes.bfloat16)


# ------------------------------------------------------------- bass builder
def _build_module(zero_bias):
    import concourse.bacc as bacc
    import concourse.mybir as mybir
    import concourse.tile as tile
    from concourse.masks import make_identity
    from contextlib import ExitStack

    F32 = mybir.dt.float32
    BF16 = mybir.dt.bfloat16

    nc = bacc.Bacc(
        "TRN2",
        target_bir_lowering=False,
        debug=False,
        enable_asserts=False,
        num_devices=NCORES,
    )

    # x^T pre-tiled on host: [ct, p, t] with c = ct*128 + p
    xt = nc.dram_tensor("xt", [C // P, P, TQ], BF16, kind="ExternalInput").ap()
    wa = nc.dram_tensor("wa", [C, 3 * C], BF16, kind="ExternalInput").ap()
    ba = nc.dram_tensor("ba", [3 * C], F32, kind="ExternalInput").ap()
    wp = nc.dram_tensor("wp", [C, C], BF16, kind="ExternalInput").ap()
    bp = nc.dram_tensor("bp", [C], F32, kind="ExternalInput").ap()
    mk = nc.dram_tensor("mk", [P, NMASK, P], BF16, kind="ExternalInput").ap()
    y = nc.dram_tensor("y", [CHUNK, C], BF16, kind="ExternalOutput").ap()

    Exp = mybir.ActivationFunctionType.Exp
    Ident = mybir.ActivationFunctionType.Identity
    ADD = mybir.AluOpType.add
    MULT = mybir.AluOpType.mult

    scale = 1.0 / np.sqrt(D)
    NCT = C // P               # 8 channel tiles

    with tile.TileContext(nc) as tc, ExitStack() as ctx:
        const = ctx.enter_context(tc.tile_pool(name="const", bufs=1))
        big = ctx.enter_context(tc.tile_pool(name="big", bufs=1))
        wpool = ctx.enter_context(tc.tile_pool(name="wpool", bufs=3))
        wppool = ctx.enter_context(tc.tile_pool(name="wppool", bufs=1))
        slabp = ctx.enter_context(tc.tile_pool(name="slabp", bufs=14))
        rrowp = ctx.enter_context(tc.tile_pool(name="rrowp", bufs=4))
        pairp = ctx.enter_context(tc.tile_pool(name="pairp", bufs=6))
        rcpp = ctx.enter_context(tc.tile_pool(name="rcpp", bufs=4))
        yap = ctx.enter_context(tc.tile_pool(name="yap", bufs=8))
        yout = ctx.enter_context(tc.tile_pool(name="yout", bufs=3))
        ps512 = ctx.enter_context(tc.tile_pool(name="ps512", bufs=2, space="PSUM"))
        ps384 = ctx.enter_context(tc.tile_pool(name="ps384", bufs=3, space="PSUM"))
        pspav = ctx.enter_context(tc.tile_pool(name="pspav", bufs=2, space="PSUM"))
        pstr = ctx.enter_context(tc.tile_pool(name="pstr", bufs=1, space="PSUM"))

        # ---------------- constants / biases
        if not zero_bias:
            bqk = const.tile([P, 16], F32)      # b_attn[:2048] as [128, jt]
            with nc.allow_non_contiguous_dma(reason="tiny bias rearrange"):
                nc.sync.dma_start(
                    bqk, ba[: 2 * C].rearrange("(j p) -> p j", p=P))
            bv_row = rrowp.tile([1, C], F32, tag="brow")
            nc.sync.dma_start(bv_row, ba[None, 2 * C:])
            bv_b = const.tile([P, C], F32)
            nc.gpsimd.partition_broadcast(bv_b, bv_row)
            bp_row = rrowp.tile([1, C], F32, tag="brow")
            nc.sync.dma_start(bp_row, bp[None, :])
            bp_b = const.tile([P, C], F32)
            nc.gpsimd.partition_broadcast(bp_b, bp_row)

        def load_w(col0):
            wt = wpool.tile([P, NCT, 512], BF16, tag="wchunk")
            with nc.allow_non_contiguous_dma(reason="batched W load"):
                for h in range(2):
                    nc.sync.dma_start(
                        wt[:, 4 * h:4 * h + 4, :],
                        wa[512 * h:512 * h + 512, col0:col0 + 512]
                            .rearrange("(ct p) j -> p ct j", p=P))
            return wt

        # ---------------- weight + x^T loads (weights first: Q needs both)
        wq0 = load_w(0)
        wk0 = load_w(C)
        xT = big.tile([P, NCT, TQ], BF16, tag="xT")
        for ct in range(NCT):     # split across queues for parallel bandwidth
            nc.sync.dma_start(xT[:, ct, :], xt[ct])

        masks = const.tile([P, NMASK, P], BF16)
        nc.sync.dma_start(masks, mk)
        identv = const.tile([P, P], BF16)
        make_identity(nc, identv)

        # PE warm-up: dense dummy matmuls while the first DMAs land, so the
        # HAM clock-gate ramps before real matmuls start.
        warm = const.tile([P, 512], BF16)
        nc.gpsimd.memset(warm, 0.0)
        for wi in range(20):
            wps = ps512.tile([P, 512], F32, tag="ps512", name=f"wps{wi}")
            nc.tensor.matmul(wps, warm[:, :P], warm, start=True, stop=True)

        # ---------------- big persistent tensors
        # Q^T [128j, jt, 512t(own)]  /  K^T [128j, jt, 768t]
        QT = big.tile([P, 8, CHUNK], BF16, tag="QT")
        KT = big.tile([P, 8, TQ], BF16, tag="KT")
        # V natural + ones columns: [128t, tt, head, D+2]
        VS = big.tile([P, NKT, H, D + 2], BF16, tag="VS")
        ones_h = const.tile([P, NKT * H], F32)
        nc.gpsimd.memset(ones_h, 1.0)
        nc.vector.tensor_copy(
            VS[:, :, :, D], ones_h.rearrange("p (t h) -> p t h", h=H))
        nc.vector.tensor_copy(
            VS[:, :, :, D + 1], ones_h.rearrange("p (t h) -> p t h", h=H))
        # out^T in c_in-major layout: [c_pair, hp, t]
        outT = big.tile([P, 8, CHUNK], BF16, tag="outT")

        # ---------------- phase emitters
        def emit_q(jg):
            wt = wq0 if jg == 0 else load_w(jg * 512)
            for jl in range(4):
                jt = jg * 4 + jl
                ps = ps512.tile([P, CHUNK], F32, tag="ps512")
                for ct in range(NCT):
                    nc.tensor.matmul(
                        ps,
                        wt[:, ct, jl * P:(jl + 1) * P],
                        xT[:, ct, HALO:TQ],
                        start=(ct == 0), stop=(ct == NCT - 1))
                nc.scalar.activation(
                    QT[:, jt, :], ps, Ident, scale=1.0,
                    bias=0.0 if zero_bias else bqk[:, jt:jt + 1])

        _wk = {0: wk0}

        def emit_k(jg, jls):
            if jg not in _wk:
                _wk[jg] = load_w(C + jg * 512)
            wt = _wk[jg]
            for jl in jls:
                jt = jg * 4 + jl
                for half in range(2):             # 768 = 2 x 384
                    psk = ps512.tile([P, 512], F32, tag="ps512",
                                     name=f"psk{jt}_{half}")
                    ps = psk[:, :384]
                    for ct in range(NCT):
                        nc.tensor.matmul(
                            ps,
                            wt[:, ct, jl * P:(jl + 1) * P],
                            xT[:, ct, half * 384:(half + 1) * 384],
                            start=(ct == 0), stop=(ct == NCT - 1))
                    nc.scalar.activation(
                        KT[:, jt, half * 384:(half + 1) * 384], ps, Ident,
                        scale=1.0,
                        bias=0.0 if zero_bias else bqk[:, 8 + jt: 9 + jt])

        _vw = {}

        def emit_v(vc, tts):
            if vc not in _vw:
                _vw[vc] = load_w(2 * C + vc * 512)
            wt = _vw[vc]
            for tt in tts:
                ps = ps512.tile([P, 512], F32, tag="ps512")
                for ct in range(NCT):
                    nc.tensor.matmul(
                        ps,
                        xT[:, ct, tt * P:(tt + 1) * P],
                        wt[:, ct, :],
                        start=(ct == 0), stop=(ct == NCT - 1))
                if zero_bias:
                    nc.scalar.activation(
                        VS[:, tt, vc * 8:(vc + 1) * 8, 0:D],
                        ps.rearrange("p (h d) -> p h d", d=D),
                        Ident, bias=0.0, scale=1.0)
                else:
                    nc.vector.tensor_tensor(
                        VS[:, tt, vc * 8:(vc + 1) * 8, 0:D],
                        ps.rearrange("p (h d) -> p h d", d=D),
                        bv_b[:, vc * 512:(vc + 1) * 512]
                            .rearrange("p (h d) -> p h d", d=D),
                        ADD)

        mask_by_kt = {}
        for mi, (kt, qt) in enumerate(MASK_PAIRS):
            mask_by_kt.setdefault(kt, []).append((mi, qt))

        # Attention for one head-pair hp (heads 2hp, 2hp+1).
        # slab[kt][k, q] = exp(scale * k.q) masked multiplicatively (0/1,
        # post-exp; gpsimd hh=0 / vector hh=1), then attn@V with the slab as
        # the stationary operand: pair[q, d] natural layout, per-partition
        # denominators via the V ones-columns, reciprocal + scale on vector,
        # PE transpose into out^T.
        slabs_all = {}            # hp -> [slabs_hh0, slabs_hh1]

        def emit_scores(hp):
            slabs2 = [[], []]
            slabs_all[hp] = slabs2
            for kt in range(NKT):
                qlo = max(0, kt - 2)
                qhi = min(NQT - 1, kt)
                nq = (qhi - qlo + 1) * P
                for hh in range(2):              # adjacent row-tiled matmuls
                    p0 = hh * 64
                    ps = ps384.tile([P, 384], F32, tag="ps384",
                                    name=f"st{hp}_{kt}_{hh}")
                    nc.tensor.matmul(
                        ps[:, :nq],
                        KT[p0:p0 + 64, hp, kt * P:(kt + 1) * P],
                        QT[p0:p0 + 64, hp, qlo * P: qlo * P + nq],
                        start=True, stop=True)
                    slab = slabp.tile([P, 384], BF16, tag="slab",
                                      name=f"slab{hp}_{kt}_{hh}")
                    nc.scalar.activation(slab[:, :nq], ps[:, :nq], Exp,
                                         bias=0.0, scale=float(scale))
                    eng = nc.gpsimd if hh == 0 else nc.vector
                    for mi, qt in mask_by_kt.get(kt, ()):
                        qoff = (qt - qlo) * P
                        eng.tensor_tensor(
                            slab[:, qoff:qoff + P], slab[:, qoff:qoff + P],
                            masks[:, mi, :], MULT)
                    slabs2[hh].append(slab)

        def emit_pav(hp):
            slabs2 = slabs_all.pop(hp)
            pair = [pairp.tile([P, P], BF16, tag="pair",
                               name=f"pair{hp}_{i}")
                    for i in range(NQT)]
            ptr = pstr.tile([P, NQT, P], BF16, tag="pstr",
                            name=f"ptr{hp}")
            for hh in range(2):
                h = 2 * hp + hh
                p0 = hh * 64
                pav = pspav.tile([P, NQT, D + 2], F32, tag="pav",
                                 name=f"pav{hp}_{hh}")
                for qt in range(NQT):
                    for i, kt in enumerate(range(qt, qt + 3)):
                        qoff = (qt - max(0, kt - 2)) * P
                        nc.tensor.matmul(
                            pav[:, qt, :],
                            slabs2[hh][kt][:, qoff:qoff + P],
                            VS[:, kt, h, :],
                            start=(i == 0), stop=(i == 2))
                    rcp = rcpp.tile([P, 1], F32, tag="rcp")
                    nc.vector.reciprocal(rcp, pav[:, qt, D:D + 1])
                    if qt % 2 == 0:
                        nc.vector.tensor_scalar_mul(
                            pair[qt][:, p0:p0 + 64], pav[:, qt, 0:D], rcp)
                    else:
                        nc.scalar.activation(
                            pair[qt][:, p0:p0 + 64], pav[:, qt, 0:D],
                            Ident, bias=0.0, scale=rcp)
                    if hh == 1:
                        # pair[qt] complete: transpose into out^T right away
                        nc.tensor.transpose(ptr[:, qt, :], pair[qt], identv)
                        dst = outT[:, hp, qt * P:(qt + 1) * P]
                        if qt % 2 == 0:
                            nc.vector.tensor_copy(dst, ptr[:, qt, :])
                        else:
                            nc.scalar.activation(dst, ptr[:, qt, :], Ident,
                                                 bias=0.0, scale=1.0)

        # output projection, split into an early part (hp 0-5, banked to
        # SBUF) and a tail part (hp 6-7) so only 2 of 8 accumulation steps
        # per tile wait on the last attention pair.
        ysbA = [None] * 8
        ysbA_nhp = [7, 7]         # hp count folded into the A part per oc
        wpo = []

        def emit_oproj_w():
            wt = wppool.tile([P, 8, 1024], BF16, tag="wproj")
            with nc.allow_non_contiguous_dma(reason="batched W_proj load"):
                nc.sync.dma_start(
                    wt, wp.rearrange("(hp p) j -> p hp j", p=P))
            wpo.append(wt)

        def emit_oproj_a(oc, tb):
            nhp = ysbA_nhp[oc]
            ps = ps512.tile([P, 512], F32, tag="ps512")
            for hp in range(nhp):
                nc.tensor.matmul(
                    ps,
                    outT[:, hp, tb * P:(tb + 1) * P],
                    wpo[0][:, hp, oc * 512:(oc + 1) * 512],
                    start=(hp == 0), stop=(hp == nhp - 1))
            ya = yap.tile([P, 512], F32, tag="ysbA", name=f"ya{oc}_{tb}")
            nc.scalar.activation(ya, ps, Ident, bias=0.0, scale=1.0)
            ysbA[oc * NQT + tb] = ya

        def emit_oproj_b():
            for tb in range(NQT):
                for oc in (1, 0):
                    nhp = ysbA_nhp[oc]
                    ps = ps512.tile([P, 512], F32, tag="ps512")
                    hps = list(range(nhp, 8))
                    for i, hp in enumerate(hps):
                        nc.tensor.matmul(
                            ps,
                            outT[:, hp, tb * P:(tb + 1) * P],
                            wpo[0][:, hp, oc * 512:(oc + 1) * 512],
                            start=(i == 0), stop=(i == len(hps) - 1))
                    ysb = yout.tile([P, 512], BF16, tag="ysb")
                    nc.vector.tensor_tensor(ysb, ps, ysbA[oc * NQT + tb], ADD)
                    if not zero_bias:
                        nc.gpsimd.tensor_tensor(
                            ysb, ysb, bp_b[:, oc * 512:(oc + 1) * 512], ADD)
                    nc.sync.dma_start(
                        y[tb * P:(tb + 1) * P, oc * 512:(oc + 1) * 512], ysb)

        # ---------------- schedule
        # scores(hp+1) is emitted before pav(hp) so every pav has a full
        # scores phase of slack for its exp/mask chain; Q/K/V/out-proj
        # blocks fill the remaining PE bubbles.
        emit_q(0)                 # Q j-tiles 0-3
        emit_k(0, [0, 1, 2, 3])   # K j-tiles 0-3
        emit_v(0, [0, 1, 2])      # V heads 0-7, tiles 0-2
        emit_scores(0)
        emit_v(0, [3, 4, 5])
        emit_scores(1)
        emit_pav(0)
        emit_q(1)                 # Q j-tiles 4-7
        emit_scores(2)
        emit_pav(1)
        emit_k(1, [0, 1])         # K j-tiles 4-5
        emit_scores(3)
        emit_pav(2)
        emit_k(1, [2, 3])         # K j-tiles 6-7
        emit_v(1, [0, 1, 2])      # V heads 8-15, tiles 0-2
        emit_scores(4)
        emit_pav(3)
        emit_v(1, [3, 4, 5])
        emit_oproj_w()
        emit_scores(5)
        emit_pav(4)
        emit_scores(6)
        emit_pav(5)
        emit_scores(7)
        emit_pav(6)
        for tb in range(NQT):
            emit_oproj_a(0, tb)
            emit_oproj_a(1, tb)
        emit_pav(7)
        emit_oproj_b()

    nc.compile()
    return nc


def _get_module(zero_bias):
    if zero_bias not in _MODS:
        _MODS[zero_bias] = _build_module(zero_bias)
    return _MODS[zero_bias]


# ------------------------------------------------------------- host helpers
def _mask_tiles(chunk_start: int) -> np.ndarray:
    """[128, NMASK, 128] multiplicative mask tiles (1 valid / 0 invalid)."""
    out = np.zeros((P, NMASK, P), np.float32)
    kk = np.arange(P)[:, None]
    qq = np.arange(P)[None, :]
    for mi, (kt, qt) in enumerate(MASK_PAIRS):
        key_abs = chunk_start - HALO + kt * P + kk
        q_abs = chunk_start + qt * P + qq
        valid = (key_abs <= q_abs) & (key_abs >= q_abs - WIN) & (key_abs >= 0)
        out[:, mi, :] = np.where(valid, 1.0, 0.0).astype(np.float32)
    return out


def _in_maps(x, W_attn, b_attn, W_proj, b_proj):
    bf16 = _np_bf16()
    wa = np.ascontiguousarray(np.asarray(W_attn, np.float32).astype(bf16))
    wpp = np.ascontiguousarray(np.asarray(W_proj, np.float32).astype(bf16))
    ba = np.ascontiguousarray(b_attn, np.float32)
    bpp = np.ascontiguousarray(b_proj, np.float32)
    maps = []
    for c in range(NCORES):
        b, k = divmod(c, NCORES // B)
        t0 = k * CHUNK
        xhalo = np.zeros((TQ, C), np.float32)
        lo = t0 - HALO
        src_lo = max(0, lo)
        xhalo[src_lo - lo:, :] = x[b, src_lo: t0 + CHUNK]
        # pre-transpose to [C, TQ], tiled as [ct, p, t]
        xth = np.ascontiguousarray(xhalo.T).astype(bf16).reshape(
            C // P, P, TQ)
        maps.append({
            "xt": xth,
            "wa": wa,
            "ba": ba,
            "wp": wpp,
            "bp": bpp,
            "mk": _mask_tiles(t0).astype(bf16),
        })
    return maps


def _run(inputs, trace=False, trace_kwargs=None):
    from concourse import bass_utils

    zero_bias = (not np.any(inputs["b_attn"])) and \
        (not np.any(inputs["b_proj"]))
    nc = _get_module(zero_bias)
    maps = _in_maps(**inputs)
    res = bass_utils.run_bass_kernel_spmd(
        nc, maps, core_ids=list(range(NCORES)),
        trace=trace, **(trace_kwargs or {}))
    out = np.empty((B, T, C), np.float32)
    for c in range(NCORES):
        b, k = divmod(c, NCORES // B)
        out[b, k * CHUNK:(k + 1) * CHUNK] = np.asarray(
            res.results[c]["y"], np.float32)
    return out, res


def kernel(x, W_attn, b_attn, W_proj, b_proj):
    inputs = dict(x=np.asarray(x, np.float32), W_attn=W_attn, b_attn=b_attn,
                  W_proj=W_proj, b_proj=b_proj)
    out, _ = _run(inputs)
    return out


# revision 18
# speedup vs baseline: 1.0332x; 1.0332x over previous
"""Trainium2 Bass kernel for local (windowed causal) self-attention.

Problem: B=2, T=2048, C=1024, 16 heads x 64 dim, local window 256.
Sharding: T-sharding. 8 cores = 2 batches x 4 chunks of 512 tokens.
Each core receives its 512-token chunk plus a 256-token left halo of x,
pre-transposed on the host to x^T (zero-padded for chunk 0), computes
QKV / banded attention / output projection for its own rows, and writes
a disjoint [512, 1024] slice of the output. No collectives; the host
concatenates the 8 slices.

Self-contained: hardcodes all shapes; no reads of /root/problem/*.
"""

import os

os.environ.setdefault("MYCRO_LOCAL_CACHE", "1")

import numpy as np

# ---------------------------------------------------------------- constants
B, T, C = 2, 2048, 1024
H, D = 16, 64
WIN = 256                      # local attention context
NCORES = 8
CHUNK = 512                    # queries per core
HALO = 256                     # left halo (== WIN)
TQ = CHUNK + HALO              # 768 x rows per core
P = 128

NQT = CHUNK // P               # 4 query tiles per core
NKT = TQ // P                  # 6 key tiles per core

# (kt, qt) pairs whose exp'd slab block needs a multiplicative 0/1 mask.
# kt-qt==2 -> window edge; kt-qt==0 -> causal edge; (1,0) is all-valid
# generically but fully invalid on the boundary chunk (keys < 0), included
# so every core runs an identical instruction stream.
MASK_PAIRS = [(0, 0), (1, 1), (2, 2), (3, 3),
              (1, 0),
              (2, 0), (3, 1), (4, 2), (5, 3)]
NMASK = len(MASK_PAIRS)

_MODS = {}                     # cached compiled Bass modules


def _np_bf16():
    import ml_dtypes
    return np.dtype(ml_dtypes.bfloat16)


# ------------------------------------------------------------- bass builder
def _build_module(zero_bias):
    import concourse.bacc as bacc
    import concourse.mybir as mybir
    import concourse.tile as tile
    from concourse.masks import make_identity
    from contextlib import ExitStack

    F32 = mybir.dt.float32
    BF16 = mybir.dt.bfloat16

    nc = bacc.Bacc(
        "TRN2",
        target_bir_lowering=False,
        debug=False,
        enable_asserts=False,
        num_devices=NCORES,
    )

    # x^T pre-tiled on host: [ct, p, t] with c = ct*128 + p
    xt = nc.dram_tensor("xt", [C // P, P, TQ], BF16, kind="ExternalInput").ap()
    wa = nc.dram_tensor("wa", [C, 3 * C], BF16, kind="ExternalInput").ap()
    ba = nc.dram_tensor("ba", [3 * C], F32, kind="ExternalInput").ap()
    wp = nc.dram_tensor("wp", [C, C], BF16, kind="ExternalInput").ap()
    bp = nc.dram_tensor("bp", [C], F32, kind="ExternalInput").ap()
    mk = nc.dram_tensor("mk", [P, NMASK, P], BF16, kind="ExternalInput").ap()
    y = nc.dram_tensor("y", [CHUNK, C], BF16, kind="ExternalOutput").ap()

    Exp = mybir.ActivationFunctionType.Exp
    Ident = mybir.ActivationFunctionType.Identity
    ADD = mybir.AluOpType.add
    MULT = mybir.AluOpType.mult

    scale = 1.0 / np.sqrt(D)
    NCT = C // P               # 8 channel tiles

    with tile.TileContext(nc) as tc, ExitStack() as ctx:
        const = ctx.enter_context(tc.tile_pool(name="const", bufs=1))
        big = ctx.enter_context(tc.tile_pool(name="big", bufs=1))
        wpool = ctx.enter_context(tc.tile_pool(name="wpool", bufs=3))
        wppool = ctx.enter_context(tc.tile_pool(name="wppool", bufs=1))
        slabp = ctx.enter_context(tc.tile_pool(name="slabp", bufs=14))
        rrowp = ctx.enter_context(tc.tile_pool(name="rrowp", bufs=4))
        pairp = ctx.enter_context(tc.tile_pool(name="pairp", bufs=6))
        rcpp = ctx.enter_context(tc.tile_pool(name="rcpp", bufs=4))
        yap = ctx.enter_context(tc.tile_pool(name="yap", bufs=8))
        yout = ctx.enter_context(tc.tile_pool(name="yout", bufs=3))
        ps512 = ctx.enter_context(tc.tile_pool(name="ps512", bufs=2, space="PSUM"))
        ps384 = ctx.enter_context(tc.tile_pool(name="ps384", bufs=3, space="PSUM"))
        pspav = ctx.enter_context(tc.tile_pool(name="pspav", bufs=2, space="PSUM"))
        pstr = ctx.enter_context(tc.tile_pool(name="pstr", bufs=1, space="PSUM"))

        # ---------------- constants / biases
        if not zero_bias:
            bqk = const.tile([P, 16], F32)      # b_attn[:2048] as [128, jt]
            with nc.allow_non_contiguous_dma(reason="tiny bias rearrange"):
                nc.sync.dma_start(
                    bqk, ba[: 2 * C].rearrange("(j p) -> p j", p=P))
            bv_row = rrowp.tile([1, C], F32, tag="brow")
            nc.sync.dma_start(bv_row, ba[None, 2 * C:])
            bv_b = const.tile([P, C], F32)
            nc.gpsimd.partition_broadcast(bv_b, bv_row)
            bp_row = rrowp.tile([1, C], F32, tag="brow")
            nc.sync.dma_start(bp_row, bp[None, :])
            bp_b = const.tile([P, C], F32)
            nc.gpsimd.partition_broadcast(bp_b, bp_row)

        def load_w(col0):
            wt = wpool.tile([P, NCT, 512], BF16, tag="wchunk")
            with nc.allow_non_contiguous_dma(reason="batched W load"):
                for h in range(2):
                    nc.sync.dma_start(
                        wt[:, 4 * h:4 * h + 4, :],
                        wa[512 * h:512 * h + 512, col0:col0 + 512]
                            .rearrange("(ct p) j -> p ct j", p=P))
            return wt

        # ---------------- weight + x^T loads (weights first: Q needs both)
        wq0 = load_w(0)
        wk0 = load_w(C)
        xT = big.tile([P, NCT, TQ], BF16, tag="xT")
        with nc.allow_non_contiguous_dma(reason="batched strided x^T load"):
            for cg in range(4):   # split across queues for parallel bandwidth
                nc.sync.dma_start(
                    xT[:, 2 * cg:2 * cg + 2, :],
                    xt[2 * cg:2 * cg + 2].rearrange("ct p t -> p ct t"))

        masks = const.tile([P, NMASK, P], BF16)
        nc.sync.dma_start(masks, mk)
        identv = const.tile([P, P], BF16)
        make_identity(nc, identv)

        # PE warm-up: dense dummy matmuls while the first DMAs land, so the
        # HAM clock-gate ramps before real matmuls start.
        warm = const.tile([P, 512], BF16)
        nc.gpsimd.memset(warm, 0.0)
        for wi in range(20):
            wps = ps512.tile([P, 512], F32, tag="ps512", name=f"wps{wi}")
            nc.tensor.matmul(wps, warm[:, :P], warm, start=True, stop=True)

        # ---------------- big persistent tensors
        # Q^T [128j, jt, 512t(own)]  /  K^T [128j, jt, 768t]
        QT = big.tile([P, 8, CHUNK], BF16, tag="QT")
        KT = big.tile([P, 8, TQ], BF16, tag="KT")
        # V natural + ones columns: [128t, tt, head, D+2]
        VS = big.tile([P, NKT, H, D + 2], BF16, tag="VS")
        ones_h = const.tile([P, NKT * H], F32)
        nc.gpsimd.memset(ones_h, 1.0)
        nc.vector.tensor_copy(
            VS[:, :, :, D], ones_h.rearrange("p (t h) -> p t h", h=H))
        nc.vector.tensor_copy(
            VS[:, :, :, D + 1], ones_h.rearrange("p (t h) -> p t h", h=H))
        # out^T in c_in-major layout: [c_pair, hp, t]
        outT = big.tile([P, 8, CHUNK], BF16, tag="outT")

        # ---------------- phase emitters
        def emit_q(jg):
            wt = wq0 if jg == 0 else load_w(jg * 512)
            for jl in range(4):
                jt = jg * 4 + jl
                ps = ps512.tile([P, CHUNK], F32, tag="ps512")
                for ct in range(NCT):
                    nc.tensor.matmul(
                        ps,
                        wt[:, ct, jl * P:(jl + 1) * P],
                        xT[:, ct, HALO:TQ],
                        start=(ct == 0), stop=(ct == NCT - 1))
                nc.scalar.activation(
                    QT[:, jt, :], ps, Ident, scale=1.0,
                    bias=0.0 if zero_bias else bqk[:, jt:jt + 1])

        _wk = {0: wk0}

        def emit_k(jg, jls):
            if jg not in _wk:
                _wk[jg] = load_w(C + jg * 512)
            wt = _wk[jg]
            for jl in jls:
                jt = jg * 4 + jl
                for half in range(2):             # 768 = 2 x 384
                    psk = ps512.tile([P, 512], F32, tag="ps512",
                                     name=f"psk{jt}_{half}")
                    ps = psk[:, :384]
                    for ct in range(NCT):
                        nc.tensor.matmul(
                            ps,
                            wt[:, ct, jl * P:(jl + 1) * P],
                            xT[:, ct, half * 384:(half + 1) * 384],
                            start=(ct == 0), stop=(ct == NCT - 1))
                    nc.scalar.activation(
                        KT[:, jt, half * 384:(half + 1) * 384], ps, Ident,
                        scale=1.0,
                        bias=0.0 if zero_bias else bqk[:, 8 + jt: 9 + jt])

        _vw = {}

        def emit_v(vc, tts):
            if vc not in _vw:
                _vw[vc] = load_w(2 * C + vc * 512)
            wt = _vw[vc]
            for tt in tts:
                ps = ps512.tile([P, 512], F32, tag="ps512")
                for ct in range(NCT):
                    nc.tensor.matmul(
                        ps,
                        xT[:, ct, tt * P:(tt + 1) * P],
                        wt[:, ct, :],
                        start=(ct == 0), stop=(ct == NCT - 1))
                if zero_bias:
                    nc.scalar.activation(
                        VS[:, tt, vc * 8:(vc + 1) * 8, 0:D],
                        ps.rearrange("p (h d) -> p h d", d=D),
                        Ident, bias=0.0, scale=1.0)
                else:
                    nc.vector.tensor_tensor(
                        VS[:, tt, vc * 8:(vc + 1) * 8, 0:D],
                        ps.rearrange("p (h d) -> p h d", d=D),
                        bv_b[:, vc * 512:(vc + 1) * 512]
                            .rearrange("p (h d) -> p h d", d=D),
                        ADD)

        mask_by_kt = {}
        for mi, (kt, qt) in enumerate(MASK_PAIRS):
            mask_by_kt.setdefault(kt, []).append((mi, qt))

        # Attention for one head-pair hp (heads 2hp, 2hp+1).
        # slab[kt][k, q] = exp(scale * k.q) masked multiplicatively (0/1,
        # post-exp; gpsimd hh=0 / vector hh=1), then attn@V with the slab as
        # the stationary operand: pair[q, d] natural layout, per-partition
        # denominators via the V ones-columns, reciprocal + scale on vector,
        # PE transpose into out^T.
        slabs_all = {}            # hp -> [slabs_hh0, slabs_hh1]

        def emit_scores(hp):
            slabs2 = [[], []]
            slabs_all[hp] = slabs2
            for kt in range(NKT):
                qlo = max(0, kt - 2)
                qhi = min(NQT - 1, kt)
                nq = (qhi - qlo + 1) * P
                for hh in range(2):              # adjacent row-tiled matmuls
                    p0 = hh * 64
                    ps = ps384.tile([P, 384], F32, tag="ps384",
                                    name=f"st{hp}_{kt}_{hh}")
                    nc.tensor.matmul(
                        ps[:, :nq],
                        KT[p0:p0 + 64, hp, kt * P:(kt + 1) * P],
                        QT[p0:p0 + 64, hp, qlo * P: qlo * P + nq],
                        start=True, stop=True)
                    slab = slabp.tile([P, 384], BF16, tag="slab",
                                      name=f"slab{hp}_{kt}_{hh}")
                    nc.scalar.activation(slab[:, :nq], ps[:, :nq], Exp,
                                         bias=0.0, scale=float(scale))
                    eng = nc.gpsimd if hh == 0 else nc.vector
                    for mi, qt in mask_by_kt.get(kt, ()):
                        qoff = (qt - qlo) * P
                        eng.tensor_tensor(
                            slab[:, qoff:qoff + P], slab[:, qoff:qoff + P],
                            masks[:, mi, :], MULT)
                    slabs2[hh].append(slab)

        def emit_pav(hp):
            slabs2 = slabs_all.pop(hp)
            pair = [pairp.tile([P, P], BF16, tag="pair",
                               name=f"pair{hp}_{i}")
                    for i in range(NQT)]
            ptr = pstr.tile([P, NQT, P], BF16, tag="pstr",
                            name=f"ptr{hp}")
            for hh in range(2):
                h = 2 * hp + hh
                p0 = hh * 64
                pav = pspav.tile([P, NQT, D + 2], F32, tag="pav",
                                 name=f"pav{hp}_{hh}")
                for qt in range(NQT):
                    for i, kt in enumerate(range(qt, qt + 3)):
                        qoff = (qt - max(0, kt - 2)) * P
                        nc.tensor.matmul(
                            pav[:, qt, :],
                            slabs2[hh][kt][:, qoff:qoff + P],
                            VS[:, kt, h, :],
                            start=(i == 0), stop=(i == 2))
                    rcp = rcpp.tile([P, 1], F32, tag="rcp")
                    nc.vector.reciprocal(rcp, pav[:, qt, D:D + 1])
                    if qt % 2 == 0:
                        nc.vector.tensor_scalar_mul(
                            pair[qt][:, p0:p0 + 64], pav[:, qt, 0:D], rcp)
                    else:
                        nc.scalar.activation(
                            pair[qt][:, p0:p0 + 64], pav[:, qt, 0:D],
                            Ident, bias=0.0, scale=rcp)
                    if hh == 1:
                        # pair[qt] complete: transpose into out^T right away
                        nc.tensor.transpose(ptr[:, qt, :], pair[qt], identv)
                        dst = outT[:, hp, qt * P:(qt + 1) * P]
                        if qt % 2 == 0:
                            nc.vector.tensor_copy(dst, ptr[:, qt, :])
                        else:
                            nc.scalar.activation(dst, ptr[:, qt, :], Ident,
                                                 bias=0.0, scale=1.0)

        # output projection, split into an early part (hp 0-5, banked to
        # SBUF) and a tail part (hp 6-7) so only 2 of 8 accumulation steps
        # per tile wait on the last attention pair.
        ysbA = [None] * 8
        ysbA_nhp = [7, 6]         # hp count folded into the A part per oc
        wpo = []

        def emit_oproj_w():
            wt = wppool.tile([P, 8, 1024], BF16, tag="wproj")
            with nc.allow_non_contiguous_dma(reason="batched W_proj load"):
                nc.sync.dma_start(
                    wt, wp.rearrange("(hp p) j -> p hp j", p=P))
            wpo.append(wt)

        def emit_oproj_a(oc, tb):
            nhp = ysbA_nhp[oc]
            ps = ps512.tile([P, 512], F32, tag="ps512")
            for hp in range(nhp):
                nc.tensor.matmul(
                    ps,
                    outT[:, hp, tb * P:(tb + 1) * P],
                    wpo[0][:, hp, oc * 512:(oc + 1) * 512],
                    start=(hp == 0), stop=(hp == nhp - 1))
            ya = yap.tile([P, 512], F32, tag="ysbA", name=f"ya{oc}_{tb}")
            nc.scalar.activation(ya, ps, Ident, bias=0.0, scale=1.0)
            ysbA[oc * NQT + tb] = ya

        def emit_oproj_b():
            for tb in range(NQT):
                for oc in (1, 0):
                    nhp = ysbA_nhp[oc]
                    ps = ps512.tile([P, 512], F32, tag="ps512")
                    hps = list(range(nhp, 8))
                    for i, hp in enumerate(hps):
                        nc.tensor.matmul(
                            ps,
                            outT[:, hp, tb * P:(tb + 1) * P],
                            wpo[0][:, hp, oc * 512:(oc + 1) * 512],
                            start=(i == 0), stop=(i == len(hps) - 1))
                    ysb = yout.tile([P, 512], BF16, tag="ysb")
                    nc.vector.tensor_tensor(ysb, ps, ysbA[oc * NQT + tb], ADD)
                    if not zero_bias:
                        nc.gpsimd.tensor_tensor(
                            ysb, ysb, bp_b[:, oc * 512:(oc + 1) * 512], ADD)
                    nc.sync.dma_start(
                        y[tb * P:(tb + 1) * P, oc * 512:(oc + 1) * 512], ysb)

        # ---------------- schedule
        # scores(hp+1) is emitted before pav(hp) so every pav has a full
        # scores phase of slack for its exp/mask chain; Q/K/V/out-proj
        # blocks fill the remaining PE bubbles.
        emit_q(0)                 # Q j-tiles 0-3
        emit_k(0, [0, 1, 2, 3])   # K j-tiles 0-3
        emit_v(0, [0, 1, 2])      # V heads 0-7, tiles 0-2
        emit_scores(0)
        emit_v(0, [3, 4, 5])
        emit_scores(1)
        emit_pav(0)
        emit_q(1)                 # Q j-tiles 4-7
        emit_scores(2)
        emit_pav(1)
        emit_k(1, [0, 1])         # K j-tiles 4-5
        emit_scores(3)
        emit_pav(2)
        emit_k(1, [2, 3])         # K j-tiles 6-7
        emit_v(1, [0, 1, 2])      # V heads 8-15, tiles 0-2
        emit_scores(4)
        emit_pav(3)
        emit_v(1, [3, 4, 5])
        emit_oproj_w()
        emit_scores(5)
        emit_pav(4)
        emit_scores(6)
        emit_pav(5)
        for tb in range(NQT):
            emit_oproj_a(1, tb)
        emit_scores(7)
        emit_pav(6)
        for tb in range(NQT):
            emit_oproj_a(0, tb)
        emit_pav(7)
        emit_oproj_b()

    nc.compile()
    return nc


def _get_module(zero_bias):
    if zero_bias not in _MODS:
        _MODS[zero_bias] = _build_module(zero_bias)
    return _MODS[zero_bias]


# ------------------------------------------------------------- host helpers
def _mask_tiles(chunk_start: int) -> np.ndarray:
    """[128, NMASK, 128] multiplicative mask tiles (1 valid / 0 invalid)."""
    out = np.zeros((P, NMASK, P), np.float32)
    kk = np.arange(P)[:, None]
    qq = np.arange(P)[None, :]
    for mi, (kt, qt) in enumerate(MASK_PAIRS):
        key_abs = chunk_start - HALO + kt * P + kk
        q_abs = chunk_start + qt * P + qq
        valid = (key_abs <= q_abs) & (key_abs >= q_abs - WIN) & (key_abs >= 0)
        out[:, mi, :] = np.where(valid, 1.0, 0.0).astype(np.float32)
    return out


def _in_maps(x, W_attn, b_attn, W_proj, b_proj):
    bf16 = _np_bf16()
    wa = np.ascontiguousarray(np.asarray(W_attn, np.float32).astype(bf16))
    wpp = np.ascontiguousarray(np.asarray(W_proj, np.float32).astype(bf16))
    ba = np.ascontiguousarray(b_attn, np.float32)
    bpp = np.ascontiguousarray(b_proj, np.float32)
    maps = []
    for c in range(NCORES):
        b, k = divmod(c, NCORES // B)
        t0 = k * CHUNK
        xhalo = np.zeros((TQ, C), np.float32)
        lo = t0 - HALO
        src_lo = max(0, lo)
        xhalo[src_lo - lo:, :] = x[b, src_lo: t0 + CHUNK]
        # pre-transpose to [C, TQ], tiled as [ct, p, t]
        xth = np.ascontiguousarray(xhalo.T).astype(bf16).reshape(
            C // P, P, TQ)
        maps.append({
            "xt": xth,
            "wa": wa,
            "ba": ba,
            "wp": wpp,
            "bp": bpp,
            "mk": _mask_tiles(t0).astype(bf16),
        })
    return maps


def _run(inputs, trace=False, trace_kwargs=None):
    from concourse import bass_utils

    zero_bias = (not np.any(inputs["b_attn"])) and \
        (not np.any(inputs["b_proj"]))
    nc = _get_module(zero_bias)
    maps = _in_maps(**inputs)
    res = bass_utils.run_bass_kernel_spmd(
        nc, maps, core_ids=list(range(NCORES)),
        trace=trace, **(trace_kwargs or {}))
    out = np.empty((B, T, C), np.float32)
    for c in range(NCORES):
        b, k = divmod(c, NCORES // B)
        out[b, k * CHUNK:(k + 1) * CHUNK] = np.asarray(
            res.results[c]["y"], np.float32)
    return out, res


def kernel(x, W_attn, b_attn, W_proj, b_proj):
    inputs = dict(x=np.asarray(x, np.float32), W_attn=W_attn, b_attn=b_attn,
                  W_proj=W_proj, b_proj=b_proj)
    out, _ = _run(inputs)
    return out


# revision 19
# speedup vs baseline: 1.0589x; 1.0248x over previous
"""Trainium2 Bass kernel for local (windowed causal) self-attention.

Problem: B=2, T=2048, C=1024, 16 heads x 64 dim, local window 256.
Sharding: T-sharding. 8 cores = 2 batches x 4 chunks of 512 tokens.
Each core receives its 512-token chunk plus a 256-token left halo of x,
pre-transposed on the host to x^T (zero-padded for chunk 0), computes
QKV / banded attention / output projection for its own rows, and writes
a disjoint [512, 1024] slice of the output. No collectives; the host
concatenates the 8 slices.

Self-contained: hardcodes all shapes; no reads of /root/problem/*.
"""

import os

os.environ.setdefault("MYCRO_LOCAL_CACHE", "1")

import numpy as np

# ---------------------------------------------------------------- constants
B, T, C = 2, 2048, 1024
H, D = 16, 64
WIN = 256                      # local attention context
NCORES = 8
CHUNK = 512                    # queries per core
HALO = 256                     # left halo (== WIN)
TQ = CHUNK + HALO              # 768 x rows per core
P = 128

NQT = CHUNK // P               # 4 query tiles per core
NKT = TQ // P                  # 6 key tiles per core

# (kt, qt) pairs whose exp'd slab block needs a multiplicative 0/1 mask.
# kt-qt==2 -> window edge; kt-qt==0 -> causal edge; (1,0) is all-valid
# generically but fully invalid on the boundary chunk (keys < 0), included
# so every core runs an identical instruction stream.
MASK_PAIRS = [(0, 0), (1, 1), (2, 2), (3, 3),
              (1, 0),
              (2, 0), (3, 1), (4, 2), (5, 3)]
NMASK = len(MASK_PAIRS)

_MODS = {}                     # cached compiled Bass modules


def _np_bf16():
    import ml_dtypes
    return np.dtype(ml_dtypes.bfloat16)


# ------------------------------------------------------------- bass builder
def _build_module(zero_bias):
    import concourse.bacc as bacc
    import concourse.mybir as mybir
    import concourse.tile as tile
    from concourse.masks import make_identity
    from contextlib import ExitStack

    F32 = mybir.dt.float32
    BF16 = mybir.dt.bfloat16

    nc = bacc.Bacc(
        "TRN2",
        target_bir_lowering=False,
        debug=False,
        enable_asserts=False,
        num_devices=NCORES,
    )

    # x^T pre-tiled on host: [ct, p, t] with c = ct*128 + p
    xt = nc.dram_tensor("xt", [C // P, P, TQ], BF16, kind="ExternalInput").ap()
    wa = nc.dram_tensor("wa", [C, 3 * C], BF16, kind="ExternalInput").ap()
    ba = nc.dram_tensor("ba", [3 * C], F32, kind="ExternalInput").ap()
    wp = nc.dram_tensor("wp", [C, C], BF16, kind="ExternalInput").ap()
    bp = nc.dram_tensor("bp", [C], F32, kind="ExternalInput").ap()
    mk = nc.dram_tensor("mk", [P, NMASK, P], BF16, kind="ExternalInput").ap()
    y = nc.dram_tensor("y", [CHUNK, C], BF16, kind="ExternalOutput").ap()

    Exp = mybir.ActivationFunctionType.Exp
    Ident = mybir.ActivationFunctionType.Identity
    ADD = mybir.AluOpType.add
    MULT = mybir.AluOpType.mult

    scale = 1.0 / np.sqrt(D)
    NCT = C // P               # 8 channel tiles

    with tile.TileContext(nc) as tc, ExitStack() as ctx:
        const = ctx.enter_context(tc.tile_pool(name="const", bufs=1))
        big = ctx.enter_context(tc.tile_pool(name="big", bufs=1))
        wpool = ctx.enter_context(tc.tile_pool(name="wpool", bufs=3))
        wppool = ctx.enter_context(tc.tile_pool(name="wppool", bufs=1))
        slabp = ctx.enter_context(tc.tile_pool(name="slabp", bufs=14))
        rrowp = ctx.enter_context(tc.tile_pool(name="rrowp", bufs=4))
        pairp = ctx.enter_context(tc.tile_pool(name="pairp", bufs=6))
        rcpp = ctx.enter_context(tc.tile_pool(name="rcpp", bufs=4))
        yap = ctx.enter_context(tc.tile_pool(name="yap", bufs=8))
        yout = ctx.enter_context(tc.tile_pool(name="yout", bufs=3))
        ps512 = ctx.enter_context(tc.tile_pool(name="ps512", bufs=2, space="PSUM"))
        ps384 = ctx.enter_context(tc.tile_pool(name="ps384", bufs=3, space="PSUM"))
        pspav = ctx.enter_context(tc.tile_pool(name="pspav", bufs=2, space="PSUM"))
        pstr = ctx.enter_context(tc.tile_pool(name="pstr", bufs=1, space="PSUM"))

        # ---------------- constants / biases
        if not zero_bias:
            bqk = const.tile([P, 16], F32)      # b_attn[:2048] as [128, jt]
            with nc.allow_non_contiguous_dma(reason="tiny bias rearrange"):
                nc.sync.dma_start(
                    bqk, ba[: 2 * C].rearrange("(j p) -> p j", p=P))
            bv_row = rrowp.tile([1, C], F32, tag="brow")
            nc.sync.dma_start(bv_row, ba[None, 2 * C:])
            bv_b = const.tile([P, C], F32)
            nc.gpsimd.partition_broadcast(bv_b, bv_row)
            bp_row = rrowp.tile([1, C], F32, tag="brow")
            nc.sync.dma_start(bp_row, bp[None, :])
            bp_b = const.tile([P, C], F32)
            nc.gpsimd.partition_broadcast(bp_b, bp_row)

        def load_w(col0):
            wt = wpool.tile([P, NCT, 512], BF16, tag="wchunk")
            with nc.allow_non_contiguous_dma(reason="batched W load"):
                for h in range(2):
                    nc.sync.dma_start(
                        wt[:, 4 * h:4 * h + 4, :],
                        wa[512 * h:512 * h + 512, col0:col0 + 512]
                            .rearrange("(ct p) j -> p ct j", p=P))
            return wt

        # ---------------- weight + x^T loads (weights first: Q needs both)
        wq0 = load_w(0)
        wk0 = load_w(C)
        xT = big.tile([P, NCT, TQ], BF16, tag="xT")
        with nc.allow_non_contiguous_dma(reason="batched strided x^T load"):
            for cg in range(4):   # split across queues for parallel bandwidth
                nc.sync.dma_start(
                    xT[:, 2 * cg:2 * cg + 2, :],
                    xt[2 * cg:2 * cg + 2].rearrange("ct p t -> p ct t"))

        masks = const.tile([P, NMASK, P], BF16)
        nc.sync.dma_start(masks, mk)
        identv = const.tile([P, P], BF16)
        make_identity(nc, identv)

        # PE warm-up: dense dummy matmuls while the first DMAs land, so the
        # HAM clock-gate ramps before real matmuls start.
        warm = const.tile([P, 512], BF16)
        nc.gpsimd.memset(warm, 0.0)
        for wi in range(20):
            wps = ps512.tile([P, 512], F32, tag="ps512", name=f"wps{wi}")
            nc.tensor.matmul(wps, warm[:, :P], warm, start=True, stop=True)

        # ---------------- big persistent tensors
        # Q^T [128j, jt, 512t(own)]  /  K^T [128j, jt, 768t]
        QT = big.tile([P, 8, CHUNK], BF16, tag="QT")
        KT = big.tile([P, 8, TQ], BF16, tag="KT")
        # V natural + ones columns: [128t, tt, head, D+2]
        VS = big.tile([P, NKT, H, D + 2], BF16, tag="VS")
        ones_h = const.tile([P, NKT * H], F32)
        nc.gpsimd.memset(ones_h, 1.0)
        nc.vector.tensor_copy(
            VS[:, :, :, D], ones_h.rearrange("p (t h) -> p t h", h=H))
        nc.vector.tensor_copy(
            VS[:, :, :, D + 1], ones_h.rearrange("p (t h) -> p t h", h=H))
        # out^T in c_in-major layout: [c_pair, hp, t]
        outT = big.tile([P, 8, CHUNK], BF16, tag="outT")

        # ---------------- phase emitters
        def emit_q(jg):
            wt = wq0 if jg == 0 else load_w(jg * 512)
            for jl in range(4):
                jt = jg * 4 + jl
                ps = ps512.tile([P, CHUNK], F32, tag="ps512")
                for ct in range(NCT):
                    nc.tensor.matmul(
                        ps,
                        wt[:, ct, jl * P:(jl + 1) * P],
                        xT[:, ct, HALO:TQ],
                        start=(ct == 0), stop=(ct == NCT - 1))
                nc.scalar.activation(
                    QT[:, jt, :], ps, Ident, scale=1.0,
                    bias=0.0 if zero_bias else bqk[:, jt:jt + 1])

        _wk = {0: wk0}

        def emit_k(jg, jls):
            if jg not in _wk:
                _wk[jg] = load_w(C + jg * 512)
            wt = _wk[jg]
            for jl in jls:
                jt = jg * 4 + jl
                for half in range(2):             # 768 = 2 x 384
                    psk = ps512.tile([P, 512], F32, tag="ps512",
                                     name=f"psk{jt}_{half}")
                    ps = psk[:, :384]
                    for ct in range(NCT):
                        nc.tensor.matmul(
                            ps,
                            wt[:, ct, jl * P:(jl + 1) * P],
                            xT[:, ct, half * 384:(half + 1) * 384],
                            start=(ct == 0), stop=(ct == NCT - 1))
                    nc.scalar.activation(
                        KT[:, jt, half * 384:(half + 1) * 384], ps, Ident,
                        scale=1.0,
                        bias=0.0 if zero_bias else bqk[:, 8 + jt: 9 + jt])

        _vw = {}

        def emit_v(vc, tts):
            if vc not in _vw:
                _vw[vc] = load_w(2 * C + vc * 512)
            wt = _vw[vc]
            for tt in tts:
                ps = ps512.tile([P, 512], F32, tag="ps512")
                for ct in range(NCT):
                    nc.tensor.matmul(
                        ps,
                        xT[:, ct, tt * P:(tt + 1) * P],
                        wt[:, ct, :],
                        start=(ct == 0), stop=(ct == NCT - 1))
                if zero_bias:
                    nc.scalar.activation(
                        VS[:, tt, vc * 8:(vc + 1) * 8, 0:D],
                        ps.rearrange("p (h d) -> p h d", d=D),
                        Ident, bias=0.0, scale=1.0)
                else:
                    nc.vector.tensor_tensor(
                        VS[:, tt, vc * 8:(vc + 1) * 8, 0:D],
                        ps.rearrange("p (h d) -> p h d", d=D),
                        bv_b[:, vc * 512:(vc + 1) * 512]
                            .rearrange("p (h d) -> p h d", d=D),
                        ADD)

        mask_by_kt = {}
        for mi, (kt, qt) in enumerate(MASK_PAIRS):
            mask_by_kt.setdefault(kt, []).append((mi, qt))

        # Attention for one head-pair hp (heads 2hp, 2hp+1).
        # slab[kt][k, q] = exp(scale * k.q) masked multiplicatively (0/1,
        # post-exp; gpsimd hh=0 / vector hh=1), then attn@V with the slab as
        # the stationary operand: pair[q, d] natural layout, per-partition
        # denominators via the V ones-columns, reciprocal + scale on vector,
        # PE transpose into out^T.
        slabs_all = {}            # hp -> [slabs_hh0, slabs_hh1]

        def emit_scores(hp):
            slabs2 = [[], []]
            slabs_all[hp] = slabs2
            for kt in range(NKT):
                qlo = max(0, kt - 2)
                qhi = min(NQT - 1, kt)
                nq = (qhi - qlo + 1) * P
                for hh in range(2):              # adjacent row-tiled matmuls
                    p0 = hh * 64
                    ps = ps384.tile([P, 384], F32, tag="ps384",
                                    name=f"st{hp}_{kt}_{hh}")
                    nc.tensor.matmul(
                        ps[:, :nq],
                        KT[p0:p0 + 64, hp, kt * P:(kt + 1) * P],
                        QT[p0:p0 + 64, hp, qlo * P: qlo * P + nq],
                        start=True, stop=True)
                    slab = slabp.tile([P, 384], BF16, tag="slab",
                                      name=f"slab{hp}_{kt}_{hh}")
                    nc.scalar.activation(slab[:, :nq], ps[:, :nq], Exp,
                                         bias=0.0, scale=float(scale))
                    eng = nc.gpsimd if hh == 0 else nc.vector
                    for mi, qt in mask_by_kt.get(kt, ()):
                        qoff = (qt - qlo) * P
                        eng.tensor_tensor(
                            slab[:, qoff:qoff + P], slab[:, qoff:qoff + P],
                            masks[:, mi, :], MULT)
                    slabs2[hh].append(slab)

        def emit_pav(hp):
            slabs2 = slabs_all.pop(hp)
            pair = [pairp.tile([P, P], BF16, tag="pair",
                               name=f"pair{hp}_{i}")
                    for i in range(NQT)]
            ptr = pstr.tile([P, NQT, P], BF16, tag="pstr",
                            name=f"ptr{hp}")
            for hh in range(2):
                h = 2 * hp + hh
                p0 = hh * 64
                pav = pspav.tile([P, NQT, D + 2], F32, tag="pav",
                                 name=f"pav{hp}_{hh}")
                for qt in range(NQT):
                    for i, kt in enumerate(range(qt, qt + 3)):
                        qoff = (qt - max(0, kt - 2)) * P
                        nc.tensor.matmul(
                            pav[:, qt, :],
                            slabs2[hh][kt][:, qoff:qoff + P],
                            VS[:, kt, h, :],
                            start=(i == 0), stop=(i == 2))
                    rcp = rcpp.tile([P, 1], F32, tag="rcp")
                    nc.vector.reciprocal(rcp, pav[:, qt, D:D + 1])
                    nc.vector.tensor_scalar_mul(
                        pair[qt][:, p0:p0 + 64], pav[:, qt, 0:D], rcp)
                    if hh == 1:
                        # pair[qt] complete: transpose into out^T right away
                        nc.tensor.transpose(ptr[:, qt, :], pair[qt], identv)
                        dst = outT[:, hp, qt * P:(qt + 1) * P]
                        if qt % 2 == 0:
                            nc.vector.tensor_copy(dst, ptr[:, qt, :])
                        else:
                            nc.scalar.activation(dst, ptr[:, qt, :], Ident,
                                                 bias=0.0, scale=1.0)

        # output projection, split into an early part (hp 0-5, banked to
        # SBUF) and a tail part (hp 6-7) so only 2 of 8 accumulation steps
        # per tile wait on the last attention pair.
        ysbA = [None] * 8
        ysbA_nhp = [7, 6]         # hp count folded into the A part per oc
        wpo = []

        def emit_oproj_w():
            wt = wppool.tile([P, 8, 1024], BF16, tag="wproj")
            with nc.allow_non_contiguous_dma(reason="batched W_proj load"):
                nc.sync.dma_start(
                    wt, wp.rearrange("(hp p) j -> p hp j", p=P))
            wpo.append(wt)

        def emit_oproj_a(oc, tb):
            nhp = ysbA_nhp[oc]
            ps = ps512.tile([P, 512], F32, tag="ps512")
            for hp in range(nhp):
                nc.tensor.matmul(
                    ps,
                    outT[:, hp, tb * P:(tb + 1) * P],
                    wpo[0][:, hp, oc * 512:(oc + 1) * 512],
                    start=(hp == 0), stop=(hp == nhp - 1))
            ya = yap.tile([P, 512], F32, tag="ysbA", name=f"ya{oc}_{tb}")
            nc.scalar.activation(ya, ps, Ident, bias=0.0, scale=1.0)
            ysbA[oc * NQT + tb] = ya

        def emit_oproj_b():
            for tb in range(NQT):
                for oc in (1, 0):
                    nhp = ysbA_nhp[oc]
                    ps = ps512.tile([P, 512], F32, tag="ps512")
                    hps = list(range(nhp, 8))
                    for i, hp in enumerate(hps):
                        nc.tensor.matmul(
                            ps,
                            outT[:, hp, tb * P:(tb + 1) * P],
                            wpo[0][:, hp, oc * 512:(oc + 1) * 512],
                            start=(i == 0), stop=(i == len(hps) - 1))
                    ysb = yout.tile([P, 512], BF16, tag="ysb")
                    nc.vector.tensor_tensor(ysb, ps, ysbA[oc * NQT + tb], ADD)
                    if not zero_bias:
                        nc.gpsimd.tensor_tensor(
                            ysb, ysb, bp_b[:, oc * 512:(oc + 1) * 512], ADD)
                    nc.sync.dma_start(
                        y[tb * P:(tb + 1) * P, oc * 512:(oc + 1) * 512], ysb)

        # ---------------- schedule
        # scores(hp+1) is emitted before pav(hp) so every pav has a full
        # scores phase of slack for its exp/mask chain; Q/K/V/out-proj
        # blocks fill the remaining PE bubbles.
        emit_q(0)                 # Q j-tiles 0-3
        emit_k(0, [0, 1, 2, 3])   # K j-tiles 0-3
        emit_v(0, [0, 1, 2])      # V heads 0-7, tiles 0-2
        emit_scores(0)
        emit_v(0, [3, 4, 5])
        emit_scores(1)
        emit_pav(0)
        emit_q(1)                 # Q j-tiles 4-7
        emit_scores(2)
        emit_pav(1)
        emit_k(1, [0, 1])         # K j-tiles 4-5
        emit_scores(3)
        emit_pav(2)
        emit_k(1, [2, 3])         # K j-tiles 6-7
        emit_v(1, [0, 1, 2])      # V heads 8-15, tiles 0-2
        emit_scores(4)
        emit_pav(3)
        emit_v(1, [3, 4, 5])
        emit_oproj_w()
        emit_scores(5)
        emit_pav(4)
        emit_scores(6)
        emit_pav(5)
        for tb in range(NQT):
            emit_oproj_a(1, tb)
        emit_scores(7)
        emit_pav(6)
        for tb in range(NQT):
            emit_oproj_a(0, tb)
        emit_pav(7)
        emit_oproj_b()

    nc.compile()
    return nc


def _get_module(zero_bias):
    if zero_bias not in _MODS:
        _MODS[zero_bias] = _build_module(zero_bias)
    return _MODS[zero_bias]


# ------------------------------------------------------------- host helpers
def _mask_tiles(chunk_start: int) -> np.ndarray:
    """[128, NMASK, 128] multiplicative mask tiles (1 valid / 0 invalid)."""
    out = np.zeros((P, NMASK, P), np.float32)
    kk = np.arange(P)[:, None]
    qq = np.arange(P)[None, :]
    for mi, (kt, qt) in enumerate(MASK_PAIRS):
        key_abs = chunk_start - HALO + kt * P + kk
        q_abs = chunk_start + qt * P + qq
        valid = (key_abs <= q_abs) & (key_abs >= q_abs - WIN) & (key_abs >= 0)
        out[:, mi, :] = np.where(valid, 1.0, 0.0).astype(np.float32)
    return out


def _in_maps(x, W_attn, b_attn, W_proj, b_proj):
    bf16 = _np_bf16()
    wa = np.ascontiguousarray(np.asarray(W_attn, np.float32).astype(bf16))
    wpp = np.ascontiguousarray(np.asarray(W_proj, np.float32).astype(bf16))
    ba = np.ascontiguousarray(b_attn, np.float32)
    bpp = np.ascontiguousarray(b_proj, np.float32)
    maps = []
    for c in range(NCORES):
        b, k = divmod(c, NCORES // B)
        t0 = k * CHUNK
        xhalo = np.zeros((TQ, C), np.float32)
        lo = t0 - HALO
        src_lo = max(0, lo)
        xhalo[src_lo - lo:, :] = x[b, src_lo: t0 + CHUNK]
        # pre-transpose to [C, TQ], tiled as [ct, p, t]
        xth = np.ascontiguousarray(xhalo.T).astype(bf16).reshape(
            C // P, P, TQ)
        maps.append({
            "xt": xth,
            "wa": wa,
            "ba": ba,
            "wp": wpp,
            "bp": bpp,
            "mk": _mask_tiles(t0).astype(bf16),
        })
    return maps


def _run(inputs, trace=False, trace_kwargs=None):
    from concourse import bass_utils

    zero_bias = (not np.any(inputs["b_attn"])) and \
        (not np.any(inputs["b_proj"]))
    nc = _get_module(zero_bias)
    maps = _in_maps(**inputs)
    res = bass_utils.run_bass_kernel_spmd(
        nc, maps, core_ids=list(range(NCORES)),
        trace=trace, **(trace_kwargs or {}))
    out = np.empty((B, T, C), np.float32)
    for c in range(NCORES):
        b, k = divmod(c, NCORES // B)
        out[b, k * CHUNK:(k + 1) * CHUNK] = np.asarray(
            res.results[c]["y"], np.float32)
    return out, res


def kernel(x, W_attn, b_attn, W_proj, b_proj):
    inputs = dict(x=np.asarray(x, np.float32), W_attn=W_attn, b_attn=b_attn,
                  W_proj=W_proj, b_proj=b_proj)
    out, _ = _run(inputs)
    return out


# revision 20
# speedup vs baseline: 1.0619x; 1.0029x over previous
"""Trainium2 Bass kernel for local (windowed causal) self-attention.

Problem: B=2, T=2048, C=1024, 16 heads x 64 dim, local window 256.
Sharding: T-sharding. 8 cores = 2 batches x 4 chunks of 512 tokens.
Each core receives its 512-token chunk plus a 256-token left halo of x,
pre-transposed on the host to x^T (zero-padded for chunk 0), computes
QKV / banded attention / output projection for its own rows, and writes
a disjoint [512, 1024] slice of the output. No collectives; the host
concatenates the 8 slices.

Self-contained: hardcodes all shapes; no reads of /root/problem/*.
"""

import os

os.environ.setdefault("MYCRO_LOCAL_CACHE", "1")

import numpy as np

# ---------------------------------------------------------------- constants
B, T, C = 2, 2048, 1024
H, D = 16, 64
WIN = 256                      # local attention context
NCORES = 8
CHUNK = 512                    # queries per core
HALO = 256                     # left halo (== WIN)
TQ = CHUNK + HALO              # 768 x rows per core
P = 128

NQT = CHUNK // P               # 4 query tiles per core
NKT = TQ // P                  # 6 key tiles per core

# (kt, qt) pairs whose exp'd slab block needs a multiplicative 0/1 mask.
# kt-qt==2 -> window edge; kt-qt==0 -> causal edge; (1,0) is all-valid
# generically but fully invalid on the boundary chunk (keys < 0), included
# so every core runs an identical instruction stream.
MASK_PAIRS = [(0, 0), (1, 1), (2, 2), (3, 3),
              (1, 0),
              (2, 0), (3, 1), (4, 2), (5, 3)]
NMASK = len(MASK_PAIRS)

_MODS = {}                     # cached compiled Bass modules


def _np_bf16():
    import ml_dtypes
    return np.dtype(ml_dtypes.bfloat16)


# ------------------------------------------------------------- bass builder
def _build_module(zero_bias):
    import concourse.bacc as bacc
    import concourse.mybir as mybir
    import concourse.tile as tile
    from concourse.masks import make_identity
    from contextlib import ExitStack

    F32 = mybir.dt.float32
    BF16 = mybir.dt.bfloat16

    nc = bacc.Bacc(
        "TRN2",
        target_bir_lowering=False,
        debug=False,
        enable_asserts=False,
        num_devices=NCORES,
    )

    # x^T pre-tiled on host: [ct, p, t] with c = ct*128 + p
    xt = nc.dram_tensor("xt", [C // P, P, TQ], BF16, kind="ExternalInput").ap()
    wa = nc.dram_tensor("wa", [C, 3 * C], BF16, kind="ExternalInput").ap()
    ba = nc.dram_tensor("ba", [3 * C], F32, kind="ExternalInput").ap()
    wp = nc.dram_tensor("wp", [C, C], BF16, kind="ExternalInput").ap()
    bp = nc.dram_tensor("bp", [C], F32, kind="ExternalInput").ap()
    mk = nc.dram_tensor("mk", [P, NMASK, P], BF16, kind="ExternalInput").ap()
    y = nc.dram_tensor("y", [CHUNK, C], BF16, kind="ExternalOutput").ap()

    Exp = mybir.ActivationFunctionType.Exp
    Ident = mybir.ActivationFunctionType.Identity
    ADD = mybir.AluOpType.add
    MULT = mybir.AluOpType.mult

    scale = 1.0 / np.sqrt(D)
    NCT = C // P               # 8 channel tiles

    with tile.TileContext(nc) as tc, ExitStack() as ctx:
        const = ctx.enter_context(tc.tile_pool(name="const", bufs=1))
        big = ctx.enter_context(tc.tile_pool(name="big", bufs=1))
        wpool = ctx.enter_context(tc.tile_pool(name="wpool", bufs=3))
        wppool = ctx.enter_context(tc.tile_pool(name="wppool", bufs=1))
        slabp = ctx.enter_context(tc.tile_pool(name="slabp", bufs=14))
        rrowp = ctx.enter_context(tc.tile_pool(name="rrowp", bufs=4))
        pairp = ctx.enter_context(tc.tile_pool(name="pairp", bufs=6))
        rcpp = ctx.enter_context(tc.tile_pool(name="rcpp", bufs=4))
        yap = ctx.enter_context(tc.tile_pool(name="yap", bufs=8))
        yout = ctx.enter_context(tc.tile_pool(name="yout", bufs=3))
        ps512 = ctx.enter_context(tc.tile_pool(name="ps512", bufs=3, space="PSUM"))
        ps384 = ctx.enter_context(tc.tile_pool(name="ps384", bufs=2, space="PSUM"))
        pspav = ctx.enter_context(tc.tile_pool(name="pspav", bufs=2, space="PSUM"))
        pstr = ctx.enter_context(tc.tile_pool(name="pstr", bufs=1, space="PSUM"))

        # ---------------- constants / biases
        if not zero_bias:
            bqk = const.tile([P, 16], F32)      # b_attn[:2048] as [128, jt]
            with nc.allow_non_contiguous_dma(reason="tiny bias rearrange"):
                nc.sync.dma_start(
                    bqk, ba[: 2 * C].rearrange("(j p) -> p j", p=P))
            bv_row = rrowp.tile([1, C], F32, tag="brow")
            nc.sync.dma_start(bv_row, ba[None, 2 * C:])
            bv_b = const.tile([P, C], F32)
            nc.gpsimd.partition_broadcast(bv_b, bv_row)
            bp_row = rrowp.tile([1, C], F32, tag="brow")
            nc.sync.dma_start(bp_row, bp[None, :])
            bp_b = const.tile([P, C], F32)
            nc.gpsimd.partition_broadcast(bp_b, bp_row)

        def load_w(col0):
            wt = wpool.tile([P, NCT, 512], BF16, tag="wchunk")
            with nc.allow_non_contiguous_dma(reason="batched W load"):
                for h in range(2):
                    nc.sync.dma_start(
                        wt[:, 4 * h:4 * h + 4, :],
                        wa[512 * h:512 * h + 512, col0:col0 + 512]
                            .rearrange("(ct p) j -> p ct j", p=P))
            return wt

        # ---------------- weight + x^T loads (weights first: Q needs both)
        wq0 = load_w(0)
        wk0 = load_w(C)
        xT = big.tile([P, NCT, TQ], BF16, tag="xT")
        with nc.allow_non_contiguous_dma(reason="batched strided x^T load"):
            for cg in range(4):   # split across queues for parallel bandwidth
                nc.sync.dma_start(
                    xT[:, 2 * cg:2 * cg + 2, :],
                    xt[2 * cg:2 * cg + 2].rearrange("ct p t -> p ct t"))

        masks = const.tile([P, NMASK, P], BF16)
        nc.sync.dma_start(masks, mk)
        identv = const.tile([P, P], BF16)
        make_identity(nc, identv)

        # PE warm-up: dense dummy matmuls while the first DMAs land, so the
        # HAM clock-gate ramps before real matmuls start.
        warm = const.tile([P, 512], BF16)
        nc.gpsimd.memset(warm, 0.0)
        for wi in range(20):
            wps = ps512.tile([P, 512], F32, tag="ps512", name=f"wps{wi}")
            nc.tensor.matmul(wps, warm[:, :P], warm, start=True, stop=True)

        # ---------------- big persistent tensors
        # Q^T [128j, jt, 512t(own)]  /  K^T [128j, jt, 768t]
        QT = big.tile([P, 8, CHUNK], BF16, tag="QT")
        KT = big.tile([P, 8, TQ], BF16, tag="KT")
        # V natural + ones columns: [128t, tt, head, D+2]
        VS = big.tile([P, NKT, H, D + 2], BF16, tag="VS")
        ones_h = const.tile([P, NKT * H], F32)
        nc.gpsimd.memset(ones_h, 1.0)
        nc.vector.tensor_copy(
            VS[:, :, :, D], ones_h.rearrange("p (t h) -> p t h", h=H))
        nc.vector.tensor_copy(
            VS[:, :, :, D + 1], ones_h.rearrange("p (t h) -> p t h", h=H))
        # out^T in c_in-major layout: [c_pair, hp, t]
        outT = big.tile([P, 8, CHUNK], BF16, tag="outT")

        # ---------------- phase emitters
        def emit_q(jg):
            wt = wq0 if jg == 0 else load_w(jg * 512)
            for jl in range(4):
                jt = jg * 4 + jl
                ps = ps512.tile([P, CHUNK], F32, tag="ps512")
                for ct in range(NCT):
                    nc.tensor.matmul(
                        ps,
                        wt[:, ct, jl * P:(jl + 1) * P],
                        xT[:, ct, HALO:TQ],
                        start=(ct == 0), stop=(ct == NCT - 1))
                nc.scalar.activation(
                    QT[:, jt, :], ps, Ident, scale=1.0,
                    bias=0.0 if zero_bias else bqk[:, jt:jt + 1])

        _wk = {0: wk0}

        def emit_k(jg, jls):
            if jg not in _wk:
                _wk[jg] = load_w(C + jg * 512)
            wt = _wk[jg]
            for jl in jls:
                jt = jg * 4 + jl
                for half in range(2):             # 768 = 2 x 384
                    psk = ps512.tile([P, 512], F32, tag="ps512",
                                     name=f"psk{jt}_{half}")
                    ps = psk[:, :384]
                    for ct in range(NCT):
                        nc.tensor.matmul(
                            ps,
                            wt[:, ct, jl * P:(jl + 1) * P],
                            xT[:, ct, half * 384:(half + 1) * 384],
                            start=(ct == 0), stop=(ct == NCT - 1))
                    nc.scalar.activation(
                        KT[:, jt, half * 384:(half + 1) * 384], ps, Ident,
                        scale=1.0,
                        bias=0.0 if zero_bias else bqk[:, 8 + jt: 9 + jt])

        _vw = {}

        def emit_v(vc, tts):
            if vc not in _vw:
                _vw[vc] = load_w(2 * C + vc * 512)
            wt = _vw[vc]
            for tt in tts:
                ps = ps512.tile([P, 512], F32, tag="ps512")
                for ct in range(NCT):
                    nc.tensor.matmul(
                        ps,
                        xT[:, ct, tt * P:(tt + 1) * P],
                        wt[:, ct, :],
                        start=(ct == 0), stop=(ct == NCT - 1))
                if zero_bias:
                    if vc == 0:
                        nc.scalar.activation(
                            VS[:, tt, vc * 8:(vc + 1) * 8, 0:D],
                            ps.rearrange("p (h d) -> p h d", d=D),
                            Ident, bias=0.0, scale=1.0)
                    else:
                        nc.vector.tensor_copy(
                            VS[:, tt, vc * 8:(vc + 1) * 8, 0:D],
                            ps.rearrange("p (h d) -> p h d", d=D))
                else:
                    nc.vector.tensor_tensor(
                        VS[:, tt, vc * 8:(vc + 1) * 8, 0:D],
                        ps.rearrange("p (h d) -> p h d", d=D),
                        bv_b[:, vc * 512:(vc + 1) * 512]
                            .rearrange("p (h d) -> p h d", d=D),
                        ADD)

        mask_by_kt = {}
        for mi, (kt, qt) in enumerate(MASK_PAIRS):
            mask_by_kt.setdefault(kt, []).append((mi, qt))

        # Attention for one head-pair hp (heads 2hp, 2hp+1).
        # slab[kt][k, q] = exp(scale * k.q) masked multiplicatively (0/1,
        # post-exp; gpsimd hh=0 / vector hh=1), then attn@V with the slab as
        # the stationary operand: pair[q, d] natural layout, per-partition
        # denominators via the V ones-columns, reciprocal + scale on vector,
        # PE transpose into out^T.
        slabs_all = {}            # hp -> [slabs_hh0, slabs_hh1]

        def emit_scores(hp):
            slabs2 = [[], []]
            slabs_all[hp] = slabs2
            for kt in range(NKT):
                qlo = max(0, kt - 2)
                qhi = min(NQT - 1, kt)
                nq = (qhi - qlo + 1) * P
                for hh in range(2):              # adjacent row-tiled matmuls
                    p0 = hh * 64
                    ps = ps384.tile([P, 384], F32, tag="ps384",
                                    name=f"st{hp}_{kt}_{hh}")
                    nc.tensor.matmul(
                        ps[:, :nq],
                        KT[p0:p0 + 64, hp, kt * P:(kt + 1) * P],
                        QT[p0:p0 + 64, hp, qlo * P: qlo * P + nq],
                        start=True, stop=True)
                    slab = slabp.tile([P, 384], BF16, tag="slab",
                                      name=f"slab{hp}_{kt}_{hh}")
                    nc.scalar.activation(slab[:, :nq], ps[:, :nq], Exp,
                                         bias=0.0, scale=float(scale))
                    eng = nc.gpsimd if hh == 0 else nc.vector
                    for mi, qt in mask_by_kt.get(kt, ()):
                        qoff = (qt - qlo) * P
                        eng.tensor_tensor(
                            slab[:, qoff:qoff + P], slab[:, qoff:qoff + P],
                            masks[:, mi, :], MULT)
                    slabs2[hh].append(slab)

        def emit_pav(hp):
            slabs2 = slabs_all.pop(hp)
            pair = [pairp.tile([P, P], BF16, tag="pair",
                               name=f"pair{hp}_{i}")
                    for i in range(NQT)]
            ptr = pstr.tile([P, NQT, P], BF16, tag="pstr",
                            name=f"ptr{hp}")
            for hh in range(2):
                h = 2 * hp + hh
                p0 = hh * 64
                pav = pspav.tile([P, NQT, D + 2], F32, tag="pav",
                                 name=f"pav{hp}_{hh}")
                for qt in range(NQT):
                    for i, kt in enumerate(range(qt, qt + 3)):
                        qoff = (qt - max(0, kt - 2)) * P
                        nc.tensor.matmul(
                            pav[:, qt, :],
                            slabs2[hh][kt][:, qoff:qoff + P],
                            VS[:, kt, h, :],
                            start=(i == 0), stop=(i == 2))
                    rcp = rcpp.tile([P, 1], F32, tag="rcp")
                    nc.vector.reciprocal(rcp, pav[:, qt, D:D + 1])
                    nc.vector.tensor_scalar_mul(
                        pair[qt][:, p0:p0 + 64], pav[:, qt, 0:D], rcp)
                    if hh == 1:
                        # pair[qt] complete: transpose into out^T right away
                        nc.tensor.transpose(ptr[:, qt, :], pair[qt], identv)
                        dst = outT[:, hp, qt * P:(qt + 1) * P]
                        if qt % 2 == 0:
                            nc.vector.tensor_copy(dst, ptr[:, qt, :])
                        else:
                            nc.scalar.activation(dst, ptr[:, qt, :], Ident,
                                                 bias=0.0, scale=1.0)

        # output projection, split into an early part (hp 0-5, banked to
        # SBUF) and a tail part (hp 6-7) so only 2 of 8 accumulation steps
        # per tile wait on the last attention pair.
        ysbA = [None] * 8
        ysbA_nhp = [7, 6]         # hp count folded into the A part per oc
        wpo = []

        def emit_oproj_w():
            wt = wppool.tile([P, 8, 1024], BF16, tag="wproj")
            with nc.allow_non_contiguous_dma(reason="batched W_proj load"):
                nc.sync.dma_start(
                    wt, wp.rearrange("(hp p) j -> p hp j", p=P))
            wpo.append(wt)

        def emit_oproj_a(oc, tb):
            nhp = ysbA_nhp[oc]
            ps = ps512.tile([P, 512], F32, tag="ps512")
            for hp in range(nhp):
                nc.tensor.matmul(
                    ps,
                    outT[:, hp, tb * P:(tb + 1) * P],
                    wpo[0][:, hp, oc * 512:(oc + 1) * 512],
                    start=(hp == 0), stop=(hp == nhp - 1))
            ya = yap.tile([P, 512], F32, tag="ysbA", name=f"ya{oc}_{tb}")
            nc.scalar.activation(ya, ps, Ident, bias=0.0, scale=1.0)
            ysbA[oc * NQT + tb] = ya

        def emit_oproj_b():
            for tb in range(NQT):
                for oc in (1, 0):
                    nhp = ysbA_nhp[oc]
                    ps = ps512.tile([P, 512], F32, tag="ps512")
                    hps = list(range(nhp, 8))
                    for i, hp in enumerate(hps):
                        nc.tensor.matmul(
                            ps,
                            outT[:, hp, tb * P:(tb + 1) * P],
                            wpo[0][:, hp, oc * 512:(oc + 1) * 512],
                            start=(i == 0), stop=(i == len(hps) - 1))
                    ysb = yout.tile([P, 512], BF16, tag="ysb")
                    nc.vector.tensor_tensor(ysb, ps, ysbA[oc * NQT + tb], ADD)
                    if not zero_bias:
                        nc.gpsimd.tensor_tensor(
                            ysb, ysb, bp_b[:, oc * 512:(oc + 1) * 512], ADD)
                    nc.sync.dma_start(
                        y[tb * P:(tb + 1) * P, oc * 512:(oc + 1) * 512], ysb)

        # ---------------- schedule
        # scores(hp+1) is emitted before pav(hp) so every pav has a full
        # scores phase of slack for its exp/mask chain; Q/K/V/out-proj
        # blocks fill the remaining PE bubbles.
        emit_q(0)                 # Q j-tiles 0-3
        emit_k(0, [0, 1, 2, 3])   # K j-tiles 0-3
        emit_v(0, [0, 1, 2])      # V heads 0-7, tiles 0-2
        emit_scores(0)
        emit_v(0, [3, 4, 5])
        emit_scores(1)
        emit_pav(0)
        emit_q(1)                 # Q j-tiles 4-7
        emit_scores(2)
        emit_pav(1)
        emit_k(1, [0, 1])         # K j-tiles 4-5
        emit_scores(3)
        emit_pav(2)
        emit_k(1, [2, 3])         # K j-tiles 6-7
        emit_v(1, [0, 1, 2])      # V heads 8-15, tiles 0-2
        emit_scores(4)
        emit_pav(3)
        emit_v(1, [3, 4, 5])
        emit_oproj_w()
        emit_scores(5)
        emit_pav(4)
        emit_scores(6)
        emit_pav(5)
        for tb in range(NQT):
            emit_oproj_a(1, tb)
        emit_scores(7)
        emit_pav(6)
        for tb in range(NQT):
            emit_oproj_a(0, tb)
        emit_pav(7)
        emit_oproj_b()

    nc.compile()
    return nc


def _get_module(zero_bias):
    if zero_bias not in _MODS:
        _MODS[zero_bias] = _build_module(zero_bias)
    return _MODS[zero_bias]


# ------------------------------------------------------------- host helpers
def _mask_tiles(chunk_start: int) -> np.ndarray:
    """[128, NMASK, 128] multiplicative mask tiles (1 valid / 0 invalid)."""
    out = np.zeros((P, NMASK, P), np.float32)
    kk = np.arange(P)[:, None]
    qq = np.arange(P)[None, :]
    for mi, (kt, qt) in enumerate(MASK_PAIRS):
        key_abs = chunk_start - HALO + kt * P + kk
        q_abs = chunk_start + qt * P + qq
        valid = (key_abs <= q_abs) & (key_abs >= q_abs - WIN) & (key_abs >= 0)
        out[:, mi, :] = np.where(valid, 1.0, 0.0).astype(np.float32)
    return out


def _in_maps(x, W_attn, b_attn, W_proj, b_proj):
    bf16 = _np_bf16()
    wa = np.ascontiguousarray(np.asarray(W_attn, np.float32).astype(bf16))
    wpp = np.ascontiguousarray(np.asarray(W_proj, np.float32).astype(bf16))
    ba = np.ascontiguousarray(b_attn, np.float32)
    bpp = np.ascontiguousarray(b_proj, np.float32)
    maps = []
    for c in range(NCORES):
        b, k = divmod(c, NCORES // B)
        t0 = k * CHUNK
        xhalo = np.zeros((TQ, C), np.float32)
        lo = t0 - HALO
        src_lo = max(0, lo)
        xhalo[src_lo - lo:, :] = x[b, src_lo: t0 + CHUNK]
        # pre-transpose to [C, TQ], tiled as [ct, p, t]
        xth = np.ascontiguousarray(xhalo.T).astype(bf16).reshape(
            C // P, P, TQ)
        maps.append({
            "xt": xth,
            "wa": wa,
            "ba": ba,
            "wp": wpp,
            "bp": bpp,
            "mk": _mask_tiles(t0).astype(bf16),
        })
    return maps


def _run(inputs, trace=False, trace_kwargs=None):
    from concourse import bass_utils

    zero_bias = (not np.any(inputs["b_attn"])) and \
        (not np.any(inputs["b_proj"]))
    nc = _get_module(zero_bias)
    maps = _in_maps(**inputs)
    res = bass_utils.run_bass_kernel_spmd(
        nc, maps, core_ids=list(range(NCORES)),
        trace=trace, **(trace_kwargs or {}))
    out = np.empty((B, T, C), np.float32)
    for c in range(NCORES):
        b, k = divmod(c, NCORES // B)
        out[b, k * CHUNK:(k + 1) * CHUNK] = np.asarray(
            res.results[c]["y"], np.float32)
    return out, res


def kernel(x, W_attn, b_attn, W_proj, b_proj):
    inputs = dict(x=np.asarray(x, np.float32), W_attn=W_attn, b_attn=b_attn,
                  W_proj=W_proj, b_proj=b_proj)
    out, _ = _run(inputs)
    return out


# revision 21
# speedup vs baseline: 1.1051x; 1.0407x over previous
"""Trainium2 Bass kernel for local (windowed causal) self-attention.

Problem: B=2, T=2048, C=1024, 16 heads x 64 dim, local window 256.
Sharding: T-sharding. 8 cores = 2 batches x 4 chunks of 512 tokens.
Each core receives its 512-token chunk plus a 256-token left halo of x,
pre-transposed on the host to x^T (zero-padded for chunk 0), computes
QKV / banded attention / output projection for its own rows, and writes
a disjoint [512, 1024] slice of the output. No collectives; the host
concatenates the 8 slices.

Self-contained: hardcodes all shapes; no reads of /root/problem/*.
"""

import os

os.environ.setdefault("MYCRO_LOCAL_CACHE", "1")

import numpy as np

# ---------------------------------------------------------------- constants
B, T, C = 2, 2048, 1024
H, D = 16, 64
WIN = 256                      # local attention context
NCORES = 8
CHUNK = 512                    # queries per core
HALO = 256                     # left halo (== WIN)
TQ = CHUNK + HALO              # 768 x rows per core
P = 128

NQT = CHUNK // P               # 4 query tiles per core
NKT = TQ // P                  # 6 key tiles per core

# (kt, qt) pairs whose exp'd slab block needs a multiplicative 0/1 mask.
# kt-qt==2 -> window edge; kt-qt==0 -> causal edge; (1,0) is all-valid
# generically but fully invalid on the boundary chunk (keys < 0), included
# so every core runs an identical instruction stream.
MASK_PAIRS = [(0, 0), (1, 1), (2, 2), (3, 3),
              (1, 0),
              (2, 0), (3, 1), (4, 2), (5, 3)]
NMASK = len(MASK_PAIRS)

_MODS = {}                     # cached compiled Bass modules


def _np_bf16():
    import ml_dtypes
    return np.dtype(ml_dtypes.bfloat16)


# ------------------------------------------------------------- bass builder
def _build_module(zero_bias):
    import concourse.bacc as bacc
    import concourse.mybir as mybir
    import concourse.tile as tile
    from concourse.masks import make_identity
    from contextlib import ExitStack

    F32 = mybir.dt.float32
    BF16 = mybir.dt.bfloat16

    nc = bacc.Bacc(
        "TRN2",
        target_bir_lowering=False,
        debug=False,
        enable_asserts=False,
        num_devices=NCORES,
    )

    # x^T pre-tiled on host: [ct, p, t] with c = ct*128 + p
    xt = nc.dram_tensor("xt", [C // P, P, TQ], BF16, kind="ExternalInput").ap()
    wa = nc.dram_tensor("wa", [C, 3 * C], BF16, kind="ExternalInput").ap()
    ba = nc.dram_tensor("ba", [3 * C], F32, kind="ExternalInput").ap()
    wp = nc.dram_tensor("wp", [C, C], BF16, kind="ExternalInput").ap()
    bp = nc.dram_tensor("bp", [C], F32, kind="ExternalInput").ap()
    mk = nc.dram_tensor("mk", [P, NMASK, P], BF16, kind="ExternalInput").ap()
    y = nc.dram_tensor("y", [CHUNK, C], BF16, kind="ExternalOutput").ap()

    Exp = mybir.ActivationFunctionType.Exp
    Ident = mybir.ActivationFunctionType.Identity
    ADD = mybir.AluOpType.add
    MULT = mybir.AluOpType.mult

    scale = 1.0 / np.sqrt(D)
    NCT = C // P               # 8 channel tiles

    with tile.TileContext(nc) as tc, ExitStack() as ctx:
        const = ctx.enter_context(tc.tile_pool(name="const", bufs=1))
        big = ctx.enter_context(tc.tile_pool(name="big", bufs=1))
        wpool = ctx.enter_context(tc.tile_pool(name="wpool", bufs=3))
        wppool = ctx.enter_context(tc.tile_pool(name="wppool", bufs=1))
        slabp = ctx.enter_context(tc.tile_pool(name="slabp", bufs=14))
        rrowp = ctx.enter_context(tc.tile_pool(name="rrowp", bufs=4))
        pairp = ctx.enter_context(tc.tile_pool(name="pairp", bufs=6))
        rcpp = ctx.enter_context(tc.tile_pool(name="rcpp", bufs=4))
        yap = ctx.enter_context(tc.tile_pool(name="yap", bufs=8))
        yout = ctx.enter_context(tc.tile_pool(name="yout", bufs=3))
        ps512 = ctx.enter_context(tc.tile_pool(name="ps512", bufs=3, space="PSUM"))
        ps384 = ctx.enter_context(tc.tile_pool(name="ps384", bufs=2, space="PSUM"))
        pspav = ctx.enter_context(tc.tile_pool(name="pspav", bufs=2, space="PSUM"))
        pstr = ctx.enter_context(tc.tile_pool(name="pstr", bufs=1, space="PSUM"))

        # ---------------- constants / biases
        if not zero_bias:
            bqk = const.tile([P, 16], F32)      # b_attn[:2048] as [128, jt]
            with nc.allow_non_contiguous_dma(reason="tiny bias rearrange"):
                nc.sync.dma_start(
                    bqk, ba[: 2 * C].rearrange("(j p) -> p j", p=P))
            bv_row = rrowp.tile([1, C], F32, tag="brow")
            nc.sync.dma_start(bv_row, ba[None, 2 * C:])
            bv_b = const.tile([P, C], F32)
            nc.gpsimd.partition_broadcast(bv_b, bv_row)
            bp_row = rrowp.tile([1, C], F32, tag="brow")
            nc.sync.dma_start(bp_row, bp[None, :])
            bp_b = const.tile([P, C], F32)
            nc.gpsimd.partition_broadcast(bp_b, bp_row)

        def load_w(col0):
            wt = wpool.tile([P, NCT, 512], BF16, tag="wchunk")
            with nc.allow_non_contiguous_dma(reason="batched W load"):
                for h in range(2):
                    nc.sync.dma_start(
                        wt[:, 4 * h:4 * h + 4, :],
                        wa[512 * h:512 * h + 512, col0:col0 + 512]
                            .rearrange("(ct p) j -> p ct j", p=P))
            return wt

        # ---------------- weight + x^T loads (Q gates on wq + xT: issue first)
        wq0 = load_w(0)
        xT = big.tile([P, NCT, TQ], BF16, tag="xT")
        with nc.allow_non_contiguous_dma(reason="batched strided x^T load"):
            for cg in range(4):   # split across queues for parallel bandwidth
                nc.sync.dma_start(
                    xT[:, 2 * cg:2 * cg + 2, :],
                    xt[2 * cg:2 * cg + 2].rearrange("ct p t -> p ct t"))
        wk0 = load_w(C)

        masks = const.tile([P, NMASK, P], BF16)
        nc.sync.dma_start(masks, mk)
        identv = const.tile([P, P], BF16)
        make_identity(nc, identv)

        # PE warm-up: dense dummy matmuls while the first DMAs land, so the
        # HAM clock-gate ramps before real matmuls start.
        warm = const.tile([P, 512], BF16)
        nc.gpsimd.memset(warm, 0.0)
        for wi in range(26):
            wps = ps512.tile([P, 512], F32, tag="ps512", name=f"wps{wi}")
            nc.tensor.matmul(wps, warm[:, :P], warm, start=True, stop=True)

        # ---------------- big persistent tensors
        # Q^T [128j, jt, 512t(own)]  /  K^T [128j, jt, 768t]
        QT = big.tile([P, 8, CHUNK], BF16, tag="QT")
        KT = big.tile([P, 8, TQ], BF16, tag="KT")
        # V natural + ones columns: [128t, tt, head, D+2]
        VS = big.tile([P, NKT, H, D + 2], BF16, tag="VS")
        ones_h = const.tile([P, NKT * H], F32)
        nc.gpsimd.memset(ones_h, 1.0)
        nc.vector.tensor_copy(
            VS[:, :, :, D], ones_h.rearrange("p (t h) -> p t h", h=H))
        nc.vector.tensor_copy(
            VS[:, :, :, D + 1], ones_h.rearrange("p (t h) -> p t h", h=H))
        # out^T in c_in-major layout: [c_pair, hp, t]
        outT = big.tile([P, 8, CHUNK], BF16, tag="outT")

        # ---------------- phase emitters
        def emit_q(jg):
            wt = wq0 if jg == 0 else load_w(jg * 512)
            for jl in range(4):
                jt = jg * 4 + jl
                ps = ps512.tile([P, CHUNK], F32, tag="ps512")
                for ct in range(NCT):
                    nc.tensor.matmul(
                        ps,
                        wt[:, ct, jl * P:(jl + 1) * P],
                        xT[:, ct, HALO:TQ],
                        start=(ct == 0), stop=(ct == NCT - 1))
                nc.scalar.activation(
                    QT[:, jt, :], ps, Ident, scale=1.0,
                    bias=0.0 if zero_bias else bqk[:, jt:jt + 1])

        _wk = {0: wk0}

        def emit_k(jg, jls):
            if jg not in _wk:
                _wk[jg] = load_w(C + jg * 512)
            wt = _wk[jg]
            for jl in jls:
                jt = jg * 4 + jl
                for half in range(2):             # 768 = 2 x 384
                    psk = ps512.tile([P, 512], F32, tag="ps512",
                                     name=f"psk{jt}_{half}")
                    ps = psk[:, :384]
                    for ct in range(NCT):
                        nc.tensor.matmul(
                            ps,
                            wt[:, ct, jl * P:(jl + 1) * P],
                            xT[:, ct, half * 384:(half + 1) * 384],
                            start=(ct == 0), stop=(ct == NCT - 1))
                    nc.scalar.activation(
                        KT[:, jt, half * 384:(half + 1) * 384], ps, Ident,
                        scale=1.0,
                        bias=0.0 if zero_bias else bqk[:, 8 + jt: 9 + jt])

        _vw = {}

        def emit_v(vc, tts):
            if vc not in _vw:
                _vw[vc] = load_w(2 * C + vc * 512)
            wt = _vw[vc]
            for tt in tts:
                ps = ps512.tile([P, 512], F32, tag="ps512")
                for ct in range(NCT):
                    nc.tensor.matmul(
                        ps,
                        xT[:, ct, tt * P:(tt + 1) * P],
                        wt[:, ct, :],
                        start=(ct == 0), stop=(ct == NCT - 1))
                if zero_bias:
                    if vc == 0:
                        nc.scalar.activation(
                            VS[:, tt, vc * 8:(vc + 1) * 8, 0:D],
                            ps.rearrange("p (h d) -> p h d", d=D),
                            Ident, bias=0.0, scale=1.0)
                    else:
                        nc.vector.tensor_copy(
                            VS[:, tt, vc * 8:(vc + 1) * 8, 0:D],
                            ps.rearrange("p (h d) -> p h d", d=D))
                else:
                    nc.vector.tensor_tensor(
                        VS[:, tt, vc * 8:(vc + 1) * 8, 0:D],
                        ps.rearrange("p (h d) -> p h d", d=D),
                        bv_b[:, vc * 512:(vc + 1) * 512]
                            .rearrange("p (h d) -> p h d", d=D),
                        ADD)

        mask_by_kt = {}
        for mi, (kt, qt) in enumerate(MASK_PAIRS):
            mask_by_kt.setdefault(kt, []).append((mi, qt))

        # Attention for one head-pair hp (heads 2hp, 2hp+1).
        # slab[kt][k, q] = exp(scale * k.q) masked multiplicatively (0/1,
        # post-exp; gpsimd hh=0 / vector hh=1), then attn@V with the slab as
        # the stationary operand: pair[q, d] natural layout, per-partition
        # denominators via the V ones-columns, reciprocal + scale on vector,
        # PE transpose into out^T.
        slabs_all = {}            # hp -> [slabs_hh0, slabs_hh1]

        def emit_scores(hp):
            slabs2 = [[], []]
            slabs_all[hp] = slabs2
            for kt in range(NKT):
                qlo = max(0, kt - 2)
                qhi = min(NQT - 1, kt)
                nq = (qhi - qlo + 1) * P
                for hh in range(2):              # adjacent row-tiled matmuls
                    p0 = hh * 64
                    ps = ps384.tile([P, 384], F32, tag="ps384",
                                    name=f"st{hp}_{kt}_{hh}")
                    nc.tensor.matmul(
                        ps[:, :nq],
                        KT[p0:p0 + 64, hp, kt * P:(kt + 1) * P],
                        QT[p0:p0 + 64, hp, qlo * P: qlo * P + nq],
                        start=True, stop=True)
                    slab = slabp.tile([P, 384], BF16, tag="slab",
                                      name=f"slab{hp}_{kt}_{hh}")
                    nc.scalar.activation(slab[:, :nq], ps[:, :nq], Exp,
                                         bias=0.0, scale=float(scale))
                    eng = nc.gpsimd if hh == 0 else nc.vector
                    for mi, qt in mask_by_kt.get(kt, ()):
                        qoff = (qt - qlo) * P
                        eng.tensor_tensor(
                            slab[:, qoff:qoff + P], slab[:, qoff:qoff + P],
                            masks[:, mi, :], MULT)
                    slabs2[hh].append(slab)

        def emit_pav(hp):
            slabs2 = slabs_all.pop(hp)
            pair = [pairp.tile([P, P], BF16, tag="pair",
                               name=f"pair{hp}_{i}")
                    for i in range(NQT)]
            ptr = pstr.tile([P, NQT, P], BF16, tag="pstr",
                            name=f"ptr{hp}")
            for hh in range(2):
                h = 2 * hp + hh
                p0 = hh * 64
                pav = pspav.tile([P, NQT, D + 2], F32, tag="pav",
                                 name=f"pav{hp}_{hh}")
                for qt in range(NQT):
                    for i, kt in enumerate(range(qt, qt + 3)):
                        qoff = (qt - max(0, kt - 2)) * P
                        nc.tensor.matmul(
                            pav[:, qt, :],
                            slabs2[hh][kt][:, qoff:qoff + P],
                            VS[:, kt, h, :],
                            start=(i == 0), stop=(i == 2))
                    rcp = rcpp.tile([P, 1], F32, tag="rcp")
                    nc.vector.reciprocal(rcp, pav[:, qt, D:D + 1])
                    nc.vector.tensor_scalar_mul(
                        pair[qt][:, p0:p0 + 64], pav[:, qt, 0:D], rcp)
                    if hh == 1:
                        # pair[qt] complete: transpose into out^T right away
                        nc.tensor.transpose(ptr[:, qt, :], pair[qt], identv)
                        dst = outT[:, hp, qt * P:(qt + 1) * P]
                        if qt % 2 == 0:
                            nc.vector.tensor_copy(dst, ptr[:, qt, :])
                        else:
                            nc.scalar.activation(dst, ptr[:, qt, :], Ident,
                                                 bias=0.0, scale=1.0)

        # output projection, split into an early part (hp 0-5, banked to
        # SBUF) and a tail part (hp 6-7) so only 2 of 8 accumulation steps
        # per tile wait on the last attention pair.
        ysbA = [None] * 8
        ysbA_nhp = [7, 6]         # hp count folded into the A part per oc
        wpo = []

        def emit_oproj_w():
            wt = wppool.tile([P, 8, 1024], BF16, tag="wproj")
            with nc.allow_non_contiguous_dma(reason="batched W_proj load"):
                nc.sync.dma_start(
                    wt, wp.rearrange("(hp p) j -> p hp j", p=P))
            wpo.append(wt)

        def emit_oproj_a(oc, tb):
            nhp = ysbA_nhp[oc]
            ps = ps512.tile([P, 512], F32, tag="ps512")
            for hp in range(nhp):
                nc.tensor.matmul(
                    ps,
                    outT[:, hp, tb * P:(tb + 1) * P],
                    wpo[0][:, hp, oc * 512:(oc + 1) * 512],
                    start=(hp == 0), stop=(hp == nhp - 1))
            ya = yap.tile([P, 512], F32, tag="ysbA", name=f"ya{oc}_{tb}")
            nc.scalar.activation(ya, ps, Ident, bias=0.0, scale=1.0)
            ysbA[oc * NQT + tb] = ya

        def emit_oproj_b():
            for tb in range(NQT):
                for oc in (1, 0):
                    nhp = ysbA_nhp[oc]
                    ps = ps512.tile([P, 512], F32, tag="ps512")
                    hps = list(range(nhp, 8))
                    for i, hp in enumerate(hps):
                        nc.tensor.matmul(
                            ps,
                            outT[:, hp, tb * P:(tb + 1) * P],
                            wpo[0][:, hp, oc * 512:(oc + 1) * 512],
                            start=(i == 0), stop=(i == len(hps) - 1))
                    ysb = yout.tile([P, 512], BF16, tag="ysb")
                    nc.vector.tensor_tensor(ysb, ps, ysbA[oc * NQT + tb], ADD)
                    if not zero_bias:
                        nc.gpsimd.tensor_tensor(
                            ysb, ysb, bp_b[:, oc * 512:(oc + 1) * 512], ADD)
                    nc.sync.dma_start(
                        y[tb * P:(tb + 1) * P, oc * 512:(oc + 1) * 512], ysb)

        # ---------------- schedule
        # scores(hp+1) is emitted before pav(hp) so every pav has a full
        # scores phase of slack for its exp/mask chain; Q/K/V/out-proj
        # blocks fill the remaining PE bubbles.
        emit_q(0)                 # Q j-tiles 0-3
        emit_k(0, [0, 1, 2, 3])   # K j-tiles 0-3
        emit_v(0, [0, 1, 2])      # V heads 0-7, tiles 0-2
        emit_scores(0)
        emit_v(0, [3, 4, 5])
        emit_scores(1)
        emit_pav(0)
        emit_q(1)                 # Q j-tiles 4-7
        emit_scores(2)
        emit_pav(1)
        emit_k(1, [0, 1])         # K j-tiles 4-5
        emit_scores(3)
        emit_pav(2)
        emit_v(1, [0, 1, 2])      # V heads 8-15, tiles 0-2
        emit_scores(4)
        emit_pav(3)
        emit_v(1, [3, 4, 5])
        emit_oproj_w()
        emit_k(1, [2])            # K j-tile 6
        emit_scores(5)
        emit_pav(4)
        emit_k(1, [3])            # K j-tile 7
        emit_scores(6)
        emit_pav(5)
        for tb in range(NQT):
            emit_oproj_a(1, tb)
        emit_scores(7)
        emit_pav(6)
        for tb in range(NQT):
            emit_oproj_a(0, tb)
        emit_pav(7)
        emit_oproj_b()

    nc.compile()
    return nc


def _get_module(zero_bias):
    if zero_bias not in _MODS:
        _MODS[zero_bias] = _build_module(zero_bias)
    return _MODS[zero_bias]


# ------------------------------------------------------------- host helpers
def _mask_tiles(chunk_start: int) -> np.ndarray:
    """[128, NMASK, 128] multiplicative mask tiles (1 valid / 0 invalid)."""
    out = np.zeros((P, NMASK, P), np.float32)
    kk = np.arange(P)[:, None]
    qq = np.arange(P)[None, :]
    for mi, (kt, qt) in enumerate(MASK_PAIRS):
        key_abs = chunk_start - HALO + kt * P + kk
        q_abs = chunk_start + qt * P + qq
        valid = (key_abs <= q_abs) & (key_abs >= q_abs - WIN) & (key_abs >= 0)
        out[:, mi, :] = np.where(valid, 1.0, 0.0).astype(np.float32)
    return out


def _in_maps(x, W_attn, b_attn, W_proj, b_proj):
    bf16 = _np_bf16()
    wa = np.ascontiguousarray(np.asarray(W_attn, np.float32).astype(bf16))
    wpp = np.ascontiguousarray(np.asarray(W_proj, np.float32).astype(bf16))
    ba = np.ascontiguousarray(b_attn, np.float32)
    bpp = np.ascontiguousarray(b_proj, np.float32)
    maps = []
    for c in range(NCORES):
        b, k = divmod(c, NCORES // B)
        t0 = k * CHUNK
        xhalo = np.zeros((TQ, C), np.float32)
        lo = t0 - HALO
        src_lo = max(0, lo)
        xhalo[src_lo - lo:, :] = x[b, src_lo: t0 + CHUNK]
        # pre-transpose to [C, TQ], tiled as [ct, p, t]
        xth = np.ascontiguousarray(xhalo.T).astype(bf16).reshape(
            C // P, P, TQ)
        maps.append({
            "xt": xth,
            "wa": wa,
            "ba": ba,
            "wp": wpp,
            "bp": bpp,
            "mk": _mask_tiles(t0).astype(bf16),
        })
    return maps


def _run(inputs, trace=False, trace_kwargs=None):
    from concourse import bass_utils

    zero_bias = (not np.any(inputs["b_attn"])) and \
        (not np.any(inputs["b_proj"]))
    nc = _get_module(zero_bias)
    maps = _in_maps(**inputs)
    res = bass_utils.run_bass_kernel_spmd(
        nc, maps, core_ids=list(range(NCORES)),
        trace=trace, **(trace_kwargs or {}))
    out = np.empty((B, T, C), np.float32)
    for c in range(NCORES):
        b, k = divmod(c, NCORES // B)
        out[b, k * CHUNK:(k + 1) * CHUNK] = np.asarray(
            res.results[c]["y"], np.float32)
    return out, res


def kernel(x, W_attn, b_attn, W_proj, b_proj):
    inputs = dict(x=np.asarray(x, np.float32), W_attn=W_attn, b_attn=b_attn,
                  W_proj=W_proj, b_proj=b_proj)
    out, _ = _run(inputs)
    return out


# revision 22
# speedup vs baseline: 1.1296x; 1.0222x over previous
"""Trainium2 Bass kernel for local (windowed causal) self-attention.

Problem: B=2, T=2048, C=1024, 16 heads x 64 dim, local window 256.
Sharding: T-sharding. 8 cores = 2 batches x 4 chunks of 512 tokens.
Each core receives its 512-token chunk plus a 256-token left halo of x,
pre-transposed on the host to x^T (zero-padded for chunk 0), computes
QKV / banded attention / output projection for its own rows, and writes
a disjoint [512, 1024] slice of the output. No collectives; the host
concatenates the 8 slices.

Self-contained: hardcodes all shapes; no reads of /root/problem/*.
"""

import os

os.environ.setdefault("MYCRO_LOCAL_CACHE", "1")

import numpy as np

# ---------------------------------------------------------------- constants
B, T, C = 2, 2048, 1024
H, D = 16, 64
WIN = 256                      # local attention context
NCORES = 8
CHUNK = 512                    # queries per core
HALO = 256                     # left halo (== WIN)
TQ = CHUNK + HALO              # 768 x rows per core
P = 128

NQT = CHUNK // P               # 4 query tiles per core
NKT = TQ // P                  # 6 key tiles per core

# (kt, qt) pairs whose exp'd slab block needs a multiplicative 0/1 mask.
# kt-qt==2 -> window edge; kt-qt==0 -> causal edge; (1,0) is all-valid
# generically but fully invalid on the boundary chunk (keys < 0), included
# so every core runs an identical instruction stream.
MASK_PAIRS = [(0, 0), (1, 1), (2, 2), (3, 3),
              (1, 0),
              (2, 0), (3, 1), (4, 2), (5, 3)]
NMASK = len(MASK_PAIRS)

_MODS = {}                     # cached compiled Bass modules


def _np_bf16():
    import ml_dtypes
    return np.dtype(ml_dtypes.bfloat16)


# ------------------------------------------------------------- bass builder
def _build_module(zero_bias):
    import concourse.bacc as bacc
    import concourse.mybir as mybir
    import concourse.tile as tile
    from concourse.masks import make_identity
    from contextlib import ExitStack

    F32 = mybir.dt.float32
    BF16 = mybir.dt.bfloat16

    nc = bacc.Bacc(
        "TRN2",
        target_bir_lowering=False,
        debug=False,
        enable_asserts=False,
        num_devices=NCORES,
    )

    # x^T pre-tiled on host: [ct, p, t] with c = ct*128 + p
    xt = nc.dram_tensor("xt", [C // P, P, TQ], BF16, kind="ExternalInput").ap()
    wa = nc.dram_tensor("wa", [C, 3 * C], BF16, kind="ExternalInput").ap()
    ba = nc.dram_tensor("ba", [3 * C], F32, kind="ExternalInput").ap()
    wp = nc.dram_tensor("wp", [C, C], BF16, kind="ExternalInput").ap()
    bp = nc.dram_tensor("bp", [C], F32, kind="ExternalInput").ap()
    mk = nc.dram_tensor("mk", [P, NMASK, P], BF16, kind="ExternalInput").ap()
    y = nc.dram_tensor("y", [CHUNK, C], BF16, kind="ExternalOutput").ap()

    Exp = mybir.ActivationFunctionType.Exp
    Ident = mybir.ActivationFunctionType.Identity
    ADD = mybir.AluOpType.add
    MULT = mybir.AluOpType.mult

    scale = 1.0 / np.sqrt(D)
    NCT = C // P               # 8 channel tiles

    with tile.TileContext(nc) as tc, ExitStack() as ctx:
        const = ctx.enter_context(tc.tile_pool(name="const", bufs=1))
        big = ctx.enter_context(tc.tile_pool(name="big", bufs=1))
        wpool = ctx.enter_context(tc.tile_pool(name="wpool", bufs=3))
        wppool = ctx.enter_context(tc.tile_pool(name="wppool", bufs=1))
        slabp = ctx.enter_context(tc.tile_pool(name="slabp", bufs=14))
        rrowp = ctx.enter_context(tc.tile_pool(name="rrowp", bufs=4))
        pairp = ctx.enter_context(tc.tile_pool(name="pairp", bufs=8))
        rcpp = ctx.enter_context(tc.tile_pool(name="rcpp", bufs=4))
        yap = ctx.enter_context(tc.tile_pool(name="yap", bufs=8))
        yout = ctx.enter_context(tc.tile_pool(name="yout", bufs=3))
        ps512 = ctx.enter_context(tc.tile_pool(name="ps512", bufs=3, space="PSUM"))
        ps384 = ctx.enter_context(tc.tile_pool(name="ps384", bufs=2, space="PSUM"))
        pspav = ctx.enter_context(tc.tile_pool(name="pspav", bufs=2, space="PSUM"))
        pstr = ctx.enter_context(tc.tile_pool(name="pstr", bufs=1, space="PSUM"))

        # ---------------- constants / biases
        if not zero_bias:
            bqk = const.tile([P, 16], F32)      # b_attn[:2048] as [128, jt]
            with nc.allow_non_contiguous_dma(reason="tiny bias rearrange"):
                nc.sync.dma_start(
                    bqk, ba[: 2 * C].rearrange("(j p) -> p j", p=P))
            bv_row = rrowp.tile([1, C], F32, tag="brow")
            nc.sync.dma_start(bv_row, ba[None, 2 * C:])
            bv_b = const.tile([P, C], F32)
            nc.gpsimd.partition_broadcast(bv_b, bv_row)
            bp_row = rrowp.tile([1, C], F32, tag="brow")
            nc.sync.dma_start(bp_row, bp[None, :])
            bp_b = const.tile([P, C], F32)
            nc.gpsimd.partition_broadcast(bp_b, bp_row)

        def load_w(col0):
            wt = wpool.tile([P, NCT, 512], BF16, tag="wchunk")
            with nc.allow_non_contiguous_dma(reason="batched W load"):
                for h in range(2):
                    nc.sync.dma_start(
                        wt[:, 4 * h:4 * h + 4, :],
                        wa[512 * h:512 * h + 512, col0:col0 + 512]
                            .rearrange("(ct p) j -> p ct j", p=P))
            return wt

        # ---------------- weight + x^T loads (Q gates on wq + xT: issue first)
        wq0 = load_w(0)
        xT = big.tile([P, NCT, TQ], BF16, tag="xT")
        with nc.allow_non_contiguous_dma(reason="batched strided x^T load"):
            for cg in range(4):   # split across queues for parallel bandwidth
                nc.sync.dma_start(
                    xT[:, 2 * cg:2 * cg + 2, :],
                    xt[2 * cg:2 * cg + 2].rearrange("ct p t -> p ct t"))
        wk0 = load_w(C)

        masks = const.tile([P, NMASK, P], BF16)
        nc.sync.dma_start(masks, mk)
        identv = const.tile([P, P], BF16)
        make_identity(nc, identv)

        # PE warm-up: dense dummy matmuls while the first DMAs land, so the
        # HAM clock-gate ramps before real matmuls start.
        warm = const.tile([P, 512], BF16)
        nc.gpsimd.memset(warm, 0.0)
        for wi in range(26):
            wps = ps512.tile([P, 512], F32, tag="ps512", name=f"wps{wi}")
            nc.tensor.matmul(wps, warm[:, :P], warm, start=True, stop=True)

        # ---------------- big persistent tensors
        # Q^T [128j, jt, 512t(own)]  /  K^T [128j, jt, 768t]
        QT = big.tile([P, 8, CHUNK], BF16, tag="QT")
        KT = big.tile([P, 8, TQ], BF16, tag="KT")
        # V natural + ones columns: [128t, tt, head, D+2]
        VS = big.tile([P, NKT, H, D + 2], BF16, tag="VS")
        ones_h = const.tile([P, NKT * H], F32)
        nc.gpsimd.memset(ones_h, 1.0)
        nc.vector.tensor_copy(
            VS[:, :, :, D], ones_h.rearrange("p (t h) -> p t h", h=H))
        nc.vector.tensor_copy(
            VS[:, :, :, D + 1], ones_h.rearrange("p (t h) -> p t h", h=H))
        # out^T in c_in-major layout: [c_pair, hp, t]
        outT = big.tile([P, 8, CHUNK], BF16, tag="outT")

        # ---------------- phase emitters
        def emit_q(jg):
            wt = wq0 if jg == 0 else load_w(jg * 512)
            for jl in range(4):
                jt = jg * 4 + jl
                ps = ps512.tile([P, CHUNK], F32, tag="ps512")
                for ct in range(NCT):
                    nc.tensor.matmul(
                        ps,
                        wt[:, ct, jl * P:(jl + 1) * P],
                        xT[:, ct, HALO:TQ],
                        start=(ct == 0), stop=(ct == NCT - 1))
                nc.scalar.activation(
                    QT[:, jt, :], ps, Ident, scale=1.0,
                    bias=0.0 if zero_bias else bqk[:, jt:jt + 1])

        _wk = {0: wk0}

        def emit_k(jg, jls):
            if jg not in _wk:
                _wk[jg] = load_w(C + jg * 512)
            wt = _wk[jg]
            for jl in jls:
                jt = jg * 4 + jl
                for half in range(2):             # 768 = 2 x 384
                    psk = ps512.tile([P, 512], F32, tag="ps512",
                                     name=f"psk{jt}_{half}")
                    ps = psk[:, :384]
                    for ct in range(NCT):
                        nc.tensor.matmul(
                            ps,
                            wt[:, ct, jl * P:(jl + 1) * P],
                            xT[:, ct, half * 384:(half + 1) * 384],
                            start=(ct == 0), stop=(ct == NCT - 1))
                    nc.scalar.activation(
                        KT[:, jt, half * 384:(half + 1) * 384], ps, Ident,
                        scale=1.0,
                        bias=0.0 if zero_bias else bqk[:, 8 + jt: 9 + jt])

        _vw = {}

        def emit_v(vc, tts):
            if vc not in _vw:
                _vw[vc] = load_w(2 * C + vc * 512)
            wt = _vw[vc]
            for tt in tts:
                ps = ps512.tile([P, 512], F32, tag="ps512")
                for ct in range(NCT):
                    nc.tensor.matmul(
                        ps,
                        xT[:, ct, tt * P:(tt + 1) * P],
                        wt[:, ct, :],
                        start=(ct == 0), stop=(ct == NCT - 1))
                if zero_bias:
                    if vc == 0:
                        nc.scalar.activation(
                            VS[:, tt, vc * 8:(vc + 1) * 8, 0:D],
                            ps.rearrange("p (h d) -> p h d", d=D),
                            Ident, bias=0.0, scale=1.0)
                    else:
                        nc.vector.tensor_copy(
                            VS[:, tt, vc * 8:(vc + 1) * 8, 0:D],
                            ps.rearrange("p (h d) -> p h d", d=D))
                else:
                    nc.vector.tensor_tensor(
                        VS[:, tt, vc * 8:(vc + 1) * 8, 0:D],
                        ps.rearrange("p (h d) -> p h d", d=D),
                        bv_b[:, vc * 512:(vc + 1) * 512]
                            .rearrange("p (h d) -> p h d", d=D),
                        ADD)

        mask_by_kt = {}
        for mi, (kt, qt) in enumerate(MASK_PAIRS):
            mask_by_kt.setdefault(kt, []).append((mi, qt))

        # Attention for one head-pair hp (heads 2hp, 2hp+1).
        # slab[kt][k, q] = exp(scale * k.q) masked multiplicatively (0/1,
        # post-exp; gpsimd hh=0 / vector hh=1), then attn@V with the slab as
        # the stationary operand: pair[q, d] natural layout, per-partition
        # denominators via the V ones-columns, reciprocal + scale on vector,
        # PE transpose into out^T.
        slabs_all = {}            # hp -> [slabs_hh0, slabs_hh1]

        def emit_scores(hp):
            slabs2 = [[], []]
            slabs_all[hp] = slabs2
            for kt in range(NKT):
                qlo = max(0, kt - 2)
                qhi = min(NQT - 1, kt)
                nq = (qhi - qlo + 1) * P
                for hh in range(2):              # adjacent row-tiled matmuls
                    p0 = hh * 64
                    ps = ps384.tile([P, 384], F32, tag="ps384",
                                    name=f"st{hp}_{kt}_{hh}")
                    nc.tensor.matmul(
                        ps[:, :nq],
                        KT[p0:p0 + 64, hp, kt * P:(kt + 1) * P],
                        QT[p0:p0 + 64, hp, qlo * P: qlo * P + nq],
                        start=True, stop=True)
                    slab = slabp.tile([P, 384], BF16, tag="slab",
                                      name=f"slab{hp}_{kt}_{hh}")
                    nc.scalar.activation(slab[:, :nq], ps[:, :nq], Exp,
                                         bias=0.0, scale=float(scale))
                    eng = nc.gpsimd if hh == 0 else nc.vector
                    for mi, qt in mask_by_kt.get(kt, ()):
                        qoff = (qt - qlo) * P
                        eng.tensor_tensor(
                            slab[:, qoff:qoff + P], slab[:, qoff:qoff + P],
                            masks[:, mi, :], MULT)
                    slabs2[hh].append(slab)

        def emit_pav(hp):
            slabs2 = slabs_all.pop(hp)
            pair = [pairp.tile([P, P], BF16, tag="pair",
                               name=f"pair{hp}_{i}")
                    for i in range(NQT)]
            ptr = pstr.tile([P, NQT, P], BF16, tag="pstr",
                            name=f"ptr{hp}")
            for hh in range(2):
                h = 2 * hp + hh
                p0 = hh * 64
                pav = pspav.tile([P, NQT, D + 2], F32, tag="pav",
                                 name=f"pav{hp}_{hh}")
                rcp = rcpp.tile([P, NQT], F32, tag="rcp")
                for qt in range(NQT):
                    for i, kt in enumerate(range(qt, qt + 3)):
                        qoff = (qt - max(0, kt - 2)) * P
                        nc.tensor.matmul(
                            pav[:, qt, :],
                            slabs2[hh][kt][:, qoff:qoff + P],
                            VS[:, kt, h, :],
                            start=(i == 0), stop=(i == 2))
                    if qt % 2 == 1:   # batch reciprocals across qt pairs
                        nc.vector.reciprocal(
                            rcp[:, qt - 1:qt + 1],
                            pav[:, qt - 1:qt + 1, D])
                        for q2 in (qt - 1, qt):
                            nc.vector.tensor_scalar_mul(
                                pair[q2][:, p0:p0 + 64], pav[:, q2, 0:D],
                                rcp[:, q2:q2 + 1])
                    if hh == 1 and qt % 2 == 1:
                        # pair pair complete: transpose into out^T right away
                        for q2 in (qt - 1, qt):
                            nc.tensor.transpose(ptr[:, q2, :], pair[q2],
                                                identv)
                            dst = outT[:, hp, q2 * P:(q2 + 1) * P]
                            if q2 % 2 == 0:
                                nc.vector.tensor_copy(dst, ptr[:, q2, :])
                            else:
                                nc.scalar.activation(dst, ptr[:, q2, :],
                                                     Ident,
                                                     bias=0.0, scale=1.0)

        # output projection, split into an early part (hp 0-5, banked to
        # SBUF) and a tail part (hp 6-7) so only 2 of 8 accumulation steps
        # per tile wait on the last attention pair.
        ysbA = [None] * 8
        ysbA_nhp = [7, 6]         # hp count folded into the A part per oc
        wpo = []

        def emit_oproj_w():
            wt = wppool.tile([P, 8, 1024], BF16, tag="wproj")
            with nc.allow_non_contiguous_dma(reason="batched W_proj load"):
                nc.sync.dma_start(
                    wt, wp.rearrange("(hp p) j -> p hp j", p=P))
            wpo.append(wt)

        def emit_oproj_a(oc, tb):
            nhp = ysbA_nhp[oc]
            ps = ps512.tile([P, 512], F32, tag="ps512")
            for hp in range(nhp):
                nc.tensor.matmul(
                    ps,
                    outT[:, hp, tb * P:(tb + 1) * P],
                    wpo[0][:, hp, oc * 512:(oc + 1) * 512],
                    start=(hp == 0), stop=(hp == nhp - 1))
            ya = yap.tile([P, 512], F32, tag="ysbA", name=f"ya{oc}_{tb}")
            nc.scalar.activation(ya, ps, Ident, bias=0.0, scale=1.0)
            ysbA[oc * NQT + tb] = ya

        def emit_oproj_b():
            for tb in range(NQT):
                for oc in (1, 0):
                    nhp = ysbA_nhp[oc]
                    ps = ps512.tile([P, 512], F32, tag="ps512")
                    hps = list(range(nhp, 8))
                    for i, hp in enumerate(hps):
                        nc.tensor.matmul(
                            ps,
                            outT[:, hp, tb * P:(tb + 1) * P],
                            wpo[0][:, hp, oc * 512:(oc + 1) * 512],
                            start=(i == 0), stop=(i == len(hps) - 1))
                    ysb = yout.tile([P, 512], BF16, tag="ysb")
                    nc.vector.tensor_tensor(ysb, ps, ysbA[oc * NQT + tb], ADD)
                    if not zero_bias:
                        nc.gpsimd.tensor_tensor(
                            ysb, ysb, bp_b[:, oc * 512:(oc + 1) * 512], ADD)
                    nc.sync.dma_start(
                        y[tb * P:(tb + 1) * P, oc * 512:(oc + 1) * 512], ysb)

        # ---------------- schedule
        # scores(hp+1) is emitted before pav(hp) so every pav has a full
        # scores phase of slack for its exp/mask chain; Q/K/V/out-proj
        # blocks fill the remaining PE bubbles.
        emit_q(0)                 # Q j-tiles 0-3
        emit_k(0, [0, 1, 2, 3])   # K j-tiles 0-3
        emit_v(0, [0, 1, 2])      # V heads 0-7, tiles 0-2
        emit_scores(0)
        emit_v(0, [3, 4, 5])
        emit_scores(1)
        emit_pav(0)
        emit_q(1)                 # Q j-tiles 4-7
        emit_scores(2)
        emit_pav(1)
        emit_k(1, [0, 1])         # K j-tiles 4-5
        emit_scores(3)
        emit_pav(2)
        emit_v(1, [0, 1, 2])      # V heads 8-15, tiles 0-2
        emit_scores(4)
        emit_pav(3)
        emit_v(1, [3, 4, 5])
        emit_oproj_w()
        emit_k(1, [2])            # K j-tile 6
        emit_scores(5)
        emit_pav(4)
        emit_k(1, [3])            # K j-tile 7
        emit_scores(6)
        emit_pav(5)
        for tb in range(NQT):
            emit_oproj_a(1, tb)
        emit_scores(7)
        emit_pav(6)
        for tb in range(NQT):
            emit_oproj_a(0, tb)
        emit_pav(7)
        emit_oproj_b()

    nc.compile()
    return nc


def _get_module(zero_bias):
    if zero_bias not in _MODS:
        _MODS[zero_bias] = _build_module(zero_bias)
    return _MODS[zero_bias]


# ------------------------------------------------------------- host helpers
def _mask_tiles(chunk_start: int) -> np.ndarray:
    """[128, NMASK, 128] multiplicative mask tiles (1 valid / 0 invalid)."""
    out = np.zeros((P, NMASK, P), np.float32)
    kk = np.arange(P)[:, None]
    qq = np.arange(P)[None, :]
    for mi, (kt, qt) in enumerate(MASK_PAIRS):
        key_abs = chunk_start - HALO + kt * P + kk
        q_abs = chunk_start + qt * P + qq
        valid = (key_abs <= q_abs) & (key_abs >= q_abs - WIN) & (key_abs >= 0)
        out[:, mi, :] = np.where(valid, 1.0, 0.0).astype(np.float32)
    return out


def _in_maps(x, W_attn, b_attn, W_proj, b_proj):
    bf16 = _np_bf16()
    wa = np.ascontiguousarray(np.asarray(W_attn, np.float32).astype(bf16))
    wpp = np.ascontiguousarray(np.asarray(W_proj, np.float32).astype(bf16))
    ba = np.ascontiguousarray(b_attn, np.float32)
    bpp = np.ascontiguousarray(b_proj, np.float32)
    maps = []
    for c in range(NCORES):
        b, k = divmod(c, NCORES // B)
        t0 = k * CHUNK
        xhalo = np.zeros((TQ, C), np.float32)
        lo = t0 - HALO
        src_lo = max(0, lo)
        xhalo[src_lo - lo:, :] = x[b, src_lo: t0 + CHUNK]
        # pre-transpose to [C, TQ], tiled as [ct, p, t]
        xth = np.ascontiguousarray(xhalo.T).astype(bf16).reshape(
            C // P, P, TQ)
        maps.append({
            "xt": xth,
            "wa": wa,
            "ba": ba,
            "wp": wpp,
            "bp": bpp,
            "mk": _mask_tiles(t0).astype(bf16),
        })
    return maps


def _run(inputs, trace=False, trace_kwargs=None):
    from concourse import bass_utils

    zero_bias = (not np.any(inputs["b_attn"])) and \
        (not np.any(inputs["b_proj"]))
    nc = _get_module(zero_bias)
    maps = _in_maps(**inputs)
    res = bass_utils.run_bass_kernel_spmd(
        nc, maps, core_ids=list(range(NCORES)),
        trace=trace, **(trace_kwargs or {}))
    out = np.empty((B, T, C), np.float32)
    for c in range(NCORES):
        b, k = divmod(c, NCORES // B)
        out[b, k * CHUNK:(k + 1) * CHUNK] = np.asarray(
            res.results[c]["y"], np.float32)
    return out, res


def kernel(x, W_attn, b_attn, W_proj, b_proj):
    inputs = dict(x=np.asarray(x, np.float32), W_attn=W_attn, b_attn=b_attn,
                  W_proj=W_proj, b_proj=b_proj)
    out, _ = _run(inputs)
    return out
